# revision 39
# baseline (speedup 1.0000x reference)
"""Tensor-parallel GQA attention forward for one TRN2 chip (8 NeuronCores).

Strategy (8-way tensor parallel over heads, no pre-attention collectives):
  - each core owns 4 q-heads + 1 kv-head and projects them DIRECTLY in the
    transposed layout from the full x (host supplies xT [128, 16, 2048]
    bf16, streamed in 512-column chunks): out[d, s] = w[:, d]^T @ xT.
    This removes the three pre-attention AllToAlls entirely - the first
    collective in the kernel (attnT redistribution) is not needed until
    attention pair-0 is done, so the multi-rank launch skew that gates the
    first collective costs nothing.
  - q is projected as separate real/imag row-blocks (qR rows = [h0r h1r
    h2r h3r], qI likewise, host-permuted weight columns), so RoPE runs as
    six 128-partition DVE multiplies against host-prepared cosT/sinT
    [freq, seq] tables; k rides the same scheme in a packed [kr ki v]
    projection
  - scores contract r- and i- halves in two accumulating K=32 matmuls per
    head (explicit tile_position row-tiling keeps head pairs concurrent);
    kR/kI are replicated x4 across partition blocks so lhsT/rhs bases line
    up; v is PE-transposed into [s, d] for the PV matmul
  - scores land transposed (S^T[k, q]) in PSUM so exp runs straight out of
    PSUM; softmax denominators come free as ones-columns in the PV matmul;
    causal masking = skipping k-tiles above the diagonal, a column
    trapezoid on the diagonal band, and a 128-wide triangle multiply
  - attention pair-0 interleaves with the tail of the projections and
    pair-1 interleaves with the even half of the output projection, so the
    PE stays dense (HAM stays at 2.4 GHz) through the ACT-paced softmax
  - an AllToAll flips head-sharded attnT to sequence-sharded; remaining
    even-half groups fill the final collective's window, then the odd half
    runs and the halves are summed
  - compute dtype bf16 (fp32 PSUM accumulation), output fp32
"""

import numpy as np

NC_CORES = 8
SEQ = 2048
DIM = 2048
HD = 64            # head dim
SC = SEQ // NC_CORES   # 256: sequence rows per core (output shard)
CH = 512           # q-chunk width for attention / projection s-chunk
NCH = SEQ // CH    # 4
KT = SEQ // 128    # 16 k-tiles
DT = DIM // 128    # 16 d-tiles

_CACHE = {}


def _build_nc():
    import concourse.bass as bass
    import concourse.mybir as mybir
    import concourse.tile as tile
    from concourse import bacc
    from concourse.masks import make_identity

    BF = mybir.dt.bfloat16
    F32 = mybir.dt.float32
    MUL = mybir.AluOpType.mult
    ADD = mybir.AluOpType.add
    SUB = mybir.AluOpType.subtract

    nc = bacc.Bacc("TRN2", target_bir_lowering=False, debug=False,
                   num_devices=NC_CORES)

    # ---- external I/O (per-core shards) ----
    xT = nc.dram_tensor("xT", [128, DT, SEQ], BF, kind="ExternalInput")
    wqR = nc.dram_tensor("wqR", [128, DT, 128], BF, kind="ExternalInput")
    wqI = nc.dram_tensor("wqI", [128, DT, 128], BF, kind="ExternalInput")
    wkv = nc.dram_tensor("wkv", [128, DT, 128], BF, kind="ExternalInput")
    wo = nc.dram_tensor("wo", [DIM, DIM], BF, kind="ExternalInput")
    cosT_in = nc.dram_tensor("cosT", [128, SEQ], BF, kind="ExternalInput")
    sinT_in = nc.dram_tensor("sinT", [128, SEQ], BF, kind="ExternalInput")
    repk_in = nc.dram_tensor("repk", [64, 128], BF, kind="ExternalInput")
    tri2_in = nc.dram_tensor("tri2", [128, 2, 128], BF, kind="ExternalInput")
    out = nc.dram_tensor("out", [SC, DIM], F32, kind="ExternalOutput")

    groups = [list(range(NC_CORES))]

    with tile.TileContext(nc) as tc:
        # DRAM bounce buffers for the output-side collectives
        a2a_in0, _ = tc.tile([NC_CORES, 128, SC], BF,
                             space=bass.MemorySpace.DRAM, name="a2a_in0")
        a2a_out0, _ = tc.tile([NC_CORES, 128, SC], BF,
                              space=bass.MemorySpace.DRAM,
                              addr_space="Shared", name="a2a_out0")
        a2a_in1, _ = tc.tile([NC_CORES, 128, SC], BF,
                             space=bass.MemorySpace.DRAM, name="a2a_in1")
        a2a_out1, _ = tc.tile([NC_CORES, 128, SC], BF,
                              space=bass.MemorySpace.DRAM,
                              addr_space="Shared", name="a2a_out1")

        with tc.tile_pool(name="persist", bufs=1) as pp, \
             tc.tile_pool(name="work", bufs=2) as wp, \
             tc.tile_pool(name="psum", bufs=2, space="PSUM") as psp:

            # weights (small: 1.5 MB total) and tables load first
            wq_sb = pp.tile([128, 2, DT, 128], BF, name="wq_sb")
            nc.sync.dma_start(wq_sb[:, 0, :, :], wqR[:])
            nc.sync.dma_start(wq_sb[:, 1, :, :], wqI[:])
            wkv_sb = pp.tile([128, DT, 128], BF, name="wkv_sb")
            nc.sync.dma_start(wkv_sb[:], wkv[:])
            cosT = pp.tile([128, SEQ], BF, name="cosT")
            sinT = pp.tile([128, SEQ], BF, name="sinT")
            nc.sync.dma_start(cosT[:], cosT_in[:])
            nc.sync.dma_start(sinT[:], sinT_in[:])
            tri2 = pp.tile([128, 2, 128], BF, name="tri2")
            nc.sync.dma_start(tri2[:], tri2_in[:])
            repk_sb = pp.tile([64, 128], BF, name="repk_sb")
            nc.sync.dma_start(repk_sb[:], repk_in[:])
            ident = pp.tile([128, 128], BF, name="ident")
            make_identity(nc, ident[:])

            # roped q per pair: rows = [h0r(32) h0i(32) h1r(32) h1i(32)]
            qP = [pp.tile([128, SEQ], BF, name=f"qP{p}") for p in range(2)]
            # roped k, [kr ki] duplicated: rows = [kr ki kr ki]
            kT2 = pp.tile([128, SEQ], BF, name="kT2")
            v_sb = pp.tile([128, KT, 2 * HD], BF, name="v_sb")
            nc.gpsimd.memset(v_sb[:, :, HD:2 * HD], 1.0)

            # ---------------- direct projections, one 512-seq chunk at a time
            # Each chunk is built as a list of small steps (~4 matmuls each)
            # so it can drip into the attention loop as PE filler.
            def chunk_steps(sc):
                ssl = slice(CH * sc, CH * sc + CH)
                state = {}
                steps = []

                def load():
                    xc = wp.tile([128, DT, CH], BF, tag="xring", bufs=2,
                                 name="xc")
                    e0 = nc.scalar if sc % 2 == 0 else nc.gpsimd
                    e1 = nc.gpsimd if sc % 2 == 0 else nc.scalar
                    e0.dma_start(xc[:, 0:4, :], xT[:, 0:4, ssl])
                    e1.dma_start(xc[:, 4:8, :], xT[:, 4:8, ssl])
                    e0.dma_start(xc[:, 8:12, :], xT[:, 8:12, ssl])
                    e1.dma_start(xc[:, 12:16, :], xT[:, 12:16, ssl])
                    state["xc"] = xc
                steps.append(load)

                # packed [v(64) | kr(32) | ki(32)] projection
                def kv_mms(i0):
                    def f():
                        if i0 == 0:
                            state["pkv"] = psp.tile([128, CH], F32, tag="ps",
                                                    bufs=3, name="pkv")
                        for dt in range(i0, i0 + 4):
                            nc.tensor.matmul(
                                state["pkv"][:], wkv_sb[:, dt, :],
                                state["xc"][:, dt, :],
                                start=(dt == 0), stop=(dt == DT - 1))
                    return f
                for i0 in range(0, DT, 4):
                    steps.append(kv_mms(i0))

                def kv_post():
                    kvraw = wp.tile([128, CH], BF, tag="kvraw", bufs=2,
                                    name="kvraw")
                    nc.scalar.copy(kvraw[:], state["pkv"][:])
                    state["kvraw"] = kvraw
                    # k RoPE on rows 64:128 (cosT's freq pattern repeats
                    # every 32 rows - read the rows matching each input's
                    # partitions; DVE TT needs equal input base partitions)
                    kro = wp.tile([64, CH], BF, tag="kro", bufs=2,
                                  name="kro")
                    kr, ki = kvraw[64:96, :], kvraw[96:128, :]
                    ta = wp.tile([32, 2, CH], BF, tag="kta", bufs=2, name="kta")
                    nc.vector.tensor_tensor(ta[:, 0, :], kr,
                                            cosT[64:96, ssl], MUL)
                    nc.vector.tensor_tensor(ta[:, 1, :], ki,
                                            sinT[96:128, ssl], MUL)
                    nc.vector.tensor_tensor(kro[0:32, :], ta[:, 0, :],
                                            ta[:, 1, :], SUB)
                    nc.vector.tensor_tensor(ta[:, 0, :], kr,
                                            sinT[64:96, ssl], MUL)
                    nc.vector.tensor_tensor(ta[:, 1, :], ki,
                                            cosT[96:128, ssl], MUL)
                    nc.vector.tensor_tensor(kro[32:64, :], ta[:, 0, :],
                                            ta[:, 1, :], ADD)
                    state["kro"] = kro
                steps.append(kv_post)

                # duplicate roped [kr ki] into rows [kr ki kr ki] via a
                # tiled-identity matmul (lane-correct partition broadcast)
                def k_rep():
                    prep = psp.tile([128, CH], F32, tag="ps", bufs=3,
                                    name="prep")
                    nc.tensor.matmul(prep[:], repk_sb[:], state["kro"][:],
                                     start=True, stop=True)
                    nc.vector.tensor_copy(kT2[:, ssl], prep[:])
                steps.append(k_rep)

                # v: PE-transpose [128, 128] blocks; keep the v rows
                def v_tr(i):
                    def f():
                        kt = 4 * sc + i
                        tv = psp.tile([128, 128], BF, tag="ps", bufs=3,
                                      name="tv")
                        nc.tensor.transpose(tv[:],
                                            state["kvraw"][:, 128 * i:128 * i + 128],
                                            ident[:])
                        nc.vector.tensor_copy(v_sb[:, kt, 0:HD],
                                              tv[0:128, 0:64])
                    return f
                for i in range(4):
                    steps.append(v_tr(i))

                # q r/i projections
                def q_mms(ri, i0):
                    def f():
                        if i0 == 0:
                            state[f"pq{ri}"] = psp.tile([128, CH], F32,
                                                        tag="ps", bufs=3,
                                                        name=f"pq{ri}")
                        for dt in range(i0, i0 + 4):
                            nc.tensor.matmul(
                                state[f"pq{ri}"][:], wq_sb[:, ri, dt, :],
                                state["xc"][:, dt, :],
                                start=(dt == 0), stop=(dt == DT - 1))
                    return f

                def q_post(ri):
                    def f():
                        qraw = wp.tile([128, CH], BF, tag=f"qraw{ri}", bufs=2,
                                       name=f"qraw{ri}")
                        nc.scalar.copy(qraw[:], state[f"pq{ri}"][:])
                        state[f"qraw{ri}"] = qraw
                    return f
                for ri in range(2):
                    for i0 in range(0, DT, 4):
                        steps.append(q_mms(ri, i0))
                    steps.append(q_post(ri))

                def q_rope():
                    # products on all 128 rows at once, then per-head 32-row
                    # writes into the pair tiles' [hr | hi] row layout
                    ct4, st4 = cosT[:, ssl], sinT[:, ssl]
                    qr, qi = state["qraw0"][:], state["qraw1"][:]
                    tq = wp.tile([128, 2, CH], BF, tag="qta", bufs=2, name="tq")
                    nc.vector.tensor_tensor(tq[:, 0, :], qr, ct4, MUL)
                    nc.vector.tensor_tensor(tq[:, 1, :], qi, st4, MUL)
                    for g in range(4):
                        rs = slice(32 * g, 32 * g + 32)
                        dst = qP[g // 2][64 * (g % 2):64 * (g % 2) + 32, ssl]
                        nc.vector.tensor_tensor(dst, tq[rs, 0, :],
                                                tq[rs, 1, :], SUB)
                    nc.vector.tensor_tensor(tq[:, 0, :], qr, st4, MUL)
                    nc.vector.tensor_tensor(tq[:, 1, :], qi, ct4, MUL)
                    for g in range(4):
                        rs = slice(32 * g, 32 * g + 32)
                        dst = qP[g // 2][64 * (g % 2) + 32:64 * (g % 2) + 64,
                                         ssl]
                        nc.vector.tensor_tensor(dst, tq[rs, 0, :],
                                                tq[rs, 1, :], ADD)
                steps.append(q_rope)
                return steps

            drip = {"steps": []}

            def drip_run(n):
                for _ in range(n):
                    if not drip["steps"]:
                        return
                    drip["steps"].pop(0)()

            def drip_flush():
                drip_run(len(drip["steps"]) + 1)

            # ---------------- attention ----------------
            attnT = pp.tile([128, 2, SEQ], BF, name="attnT")

            def attention(pair, j, interleave=None):
                nkt = 4 * j + 4
                nfull = nkt - 4
                pso0 = psp.tile([2 * HD, CH], F32, tag="ps", bufs=3, name="pso0")
                pso1 = psp.tile([2 * HD, CH], F32, tag="ps", bufs=3, name="pso1")
                qsl = slice(CH * j, CH * j + CH)
                qPt = qP[pair]

                def pv(ep_h0, ep_h1, kt, c0):
                    nc.tensor.matmul(pso0[:, c0:CH], v_sb[:, kt, :], ep_h0,
                                     start=(kt == 0), stop=(kt == nkt - 1))
                    nc.tensor.matmul(pso1[:, c0:CH], v_sb[:, kt, :], ep_h1,
                                     start=(kt == 0), stop=(kt == nkt - 1))

                # software pipeline: PV of k-tile i runs one iteration behind
                # its exp, with the drip filler issued in between on the PE
                prev = None
                for kt in range(nkt):
                    t = kt - nfull            # >= 0 on the diagonal band
                    c0 = 128 * t if t >= 0 else 0
                    ks = slice(128 * kt, 128 * kt + 128)
                    qs = slice(CH * j + c0, CH * j + CH)
                    sp = psp.tile([128, 2, CH], F32, tag="sp2", bufs=2,
                                  name="sp")
                    for h in range(2):
                        nc.tensor.matmul(sp[:, h, c0:CH],
                                         kT2[64 * h:64 * h + 64, ks],
                                         qPt[64 * h:64 * h + 64, qs],
                                         start=True, stop=True)
                    ep = wp.tile([128, 2, CH], BF, tag="exps", bufs=4,
                                 name="ep")
                    nc.scalar.activation(ep[:, :, c0:CH], sp[:, :, c0:CH],
                                         mybir.ActivationFunctionType.Exp,
                                         scale=0.125)
                    if t >= 0:
                        nc.vector.tensor_tensor(ep[:, :, c0:c0 + 128],
                                                ep[:, :, c0:c0 + 128],
                                                tri2[:], MUL)
                    if interleave is not None:
                        interleave(j, kt)
                    if prev is not None:
                        pv(*prev)
                    prev = (ep[:, 0, c0:CH], ep[:, 1, c0:CH], kt, c0)
                pv(*prev)

                for h, pso in ((0, pso0), (1, pso1)):
                    bc = wp.tile([64, CH], F32, tag="bcast", bufs=2, name="bc")
                    nc.vector.tensor_copy(bc[:], pso[HD:2 * HD, :])
                    rc = wp.tile([64, CH], F32, tag="rcp", bufs=2, name="rc")
                    nc.vector.reciprocal_approx_fast(out=rc[:], in_=bc[:])
                    nc.vector.tensor_tensor(
                        attnT[64 * h:64 * h + 64, pair, qsl],
                        pso[0:HD, :], rc[:], MUL)

            # ---------------- output projection helpers ----------------
            woA = pp.tile([128, DT // 2, DIM], BF, name="woA")
            woB = pp.tile([128, DT // 2, DIM], BF, name="woB")
            a2a_sb0 = pp.tile([128, NC_CORES, SC], BF, name="a2a_sb0")
            a2a_sb1 = pp.tile([128, NC_CORES, SC], BF, name="a2a_sb1")
            partials = pp.tile([128, 2 * NCH, CH], BF, tag="proj",
                               name="partials")
            evens = [2 * src for src in range(NC_CORES)]
            odds = [2 * src + 1 for src in range(NC_CORES)]
            chunks = [(qt, nch) for qt in range(2) for nch in range(NCH)]

            def op_mm(psf, qt, nsl, g, start, stop):
                w_ap = (woA[:, g, nsl] if g < DT // 2
                        else woB[:, g - DT // 2, nsl])
                a_ap = (a2a_sb0[:, g // 2, 128 * qt:128 * qt + 128] if g % 2 == 0
                        else a2a_sb1[:, g // 2, 128 * qt:128 * qt + 128])
                nc.tensor.matmul(psf[:], a_ap, w_ap, start=start, stop=stop)

            ev_state = {"psf": None, "n": 0}

            def even_steps(nsteps):
                for _ in range(nsteps):
                    n = ev_state["n"]
                    if n >= 64:
                        return
                    i8, i = divmod(n, NC_CORES)
                    qt, nch2 = chunks[i8]
                    if i == 0:
                        ev_state["psf"] = psp.tile([128, CH], F32, tag="psf",
                                                   bufs=1, name="psfE")
                    nsl = slice(CH * nch2, CH * nch2 + CH)
                    op_mm(ev_state["psf"], qt, nsl, evens[i],
                          i == 0, i == NC_CORES - 1)
                    if i == NC_CORES - 1:
                        nc.vector.tensor_copy(partials[:, i8, :],
                                              ev_state["psf"][:])
                    ev_state["n"] = n + 1

            # ---------------- pair-0 attention, proj-interleaved ----------
            # proj chunk c+1 drips into attention chunk c as PE filler and
            # is flushed before the attention chunk that first needs it
            for f in chunk_steps(0):
                f()
            drip_rate = (5, 3, 2, 0)

            def inter0(jj, kt):
                drip_run(drip_rate[jj])

            for j in range(NCH):
                if j < NCH - 1:
                    drip["steps"] = chunk_steps(j + 1)
                attention(0, j, interleave=inter0)
                drip_flush()
                nc.sync.dma_start(
                    a2a_in0[2 * j:2 * j + 2, :, :]
                    .rearrange("d p m -> p d m"),
                    attnT[:, 0, CH * j:CH * j + CH]
                    .rearrange("p (d m) -> p d m", m=SC))
                # anchored wo prefetch (the scheduler hoists dep-free DMAs)
                nc.vector.tensor_copy(woA[0:1, 2 * j, 0:1],
                                      attnT[0:1, 0, CH * j:CH * j + 1])
                nc.sync.dma_start(
                    woA[:, 2 * j:2 * j + 2, :],
                    wo[256 * j:256 * j + 256, :].rearrange("(t p) n -> p t n",
                                                           p=128))
                if j >= 2:   # woB too: needed by the interleaved even groups
                    jb = j - 2
                    nc.vector.tensor_copy(woB[0:1, 4 * jb, 0:1],
                                          attnT[0:1, 0, CH * j:CH * j + 1])
                    nc.gpsimd.dma_start(
                        woB[:, 4 * jb:4 * jb + 4, :],
                        wo[1024 + 512 * jb:1024 + 512 * jb + 512, :]
                        .rearrange("(t p) n -> p t n", p=128))
            nc.gpsimd.collective_compute(
                "AllToAll", mybir.AluOpType.bypass,
                replica_groups=groups, ins=[a2a_in0.opt()], outs=[a2a_out0.opt()],
            )
            for half in range(2):
                nc.sync.dma_start(
                    a2a_sb0[:, :, 128 * half:128 * half + 128],
                    a2a_out0[:, :, 128 * half:128 * half + 128]
                    .rearrange("s p m -> p s m"))

            # ---------------- pair-1 attention (pure) ----------------
            # Chunk order 1,0,2,3: chunk 0 is all-diagonal (tri-mult-gated on
            # the DVE) and would stall right behind pair-0's normalize chain.
            # No interleave: pair-1 finishes (and the final A2A triggers) as
            # early as possible; the even outproj then fills the A2A window.
            p1_order = (1, 0, 2, 3)
            for j in p1_order:
                attention(1, j)
                nc.sync.dma_start(
                    a2a_in1[2 * j:2 * j + 2, :, :]
                    .rearrange("d p m -> p d m"),
                    attnT[:, 1, CH * j:CH * j + CH]
                    .rearrange("p (d m) -> p d m", m=SC))

            # ---------------- final A2A + remaining outproj ----------------
            # evens are emitted BEFORE the collective call so tile's block
            # ordering doesn't gate them behind it - they only depend on
            # a2a_sb0, so they execute inside the collective's skew + wire
            # window on the PE
            even_steps(64)
            nc.gpsimd.collective_compute(
                "AllToAll", mybir.AluOpType.bypass,
                replica_groups=groups, ins=[a2a_in1.opt()], outs=[a2a_out1.opt()],
            )
            for half, eng in ((0, nc.sync), (1, nc.gpsimd)):
                eng.dma_start(
                    a2a_sb1[:, :, 128 * half:128 * half + 128],
                    a2a_out1[:, :, 128 * half:128 * half + 128]
                    .rearrange("s p m -> p s m"))

            store_engs = (nc.sync, nc.scalar, nc.gpsimd)
            for i8, (qt, nch2) in enumerate(chunks):
                psf = psp.tile([128, CH], F32, tag="ps", bufs=3, name="psfO")
                nsl = slice(CH * nch2, CH * nch2 + CH)
                for i, g in enumerate(odds):
                    op_mm(psf, qt, nsl, g, i == 0, i == NC_CORES - 1)
                osb = wp.tile([128, CH], F32, tag="osb", bufs=2, name="osb")
                nc.vector.tensor_tensor(osb[:], psf[:], partials[:, i8, :], ADD)
                store_engs[i8 % 3].dma_start(out[128 * qt:128 * qt + 128, nsl],
                                             osb[:])

    nc.finalize()
    return nc


def _get_nc():
    if "nc" not in _CACHE:
        _CACHE["nc"] = _build_nc()
    return _CACHE["nc"]


def _shard(inputs):
    import ml_dtypes
    x = np.ascontiguousarray(inputs["x"][0].astype(np.float32))          # [S, D]
    wq, wk, wv = (np.asarray(inputs[k]).astype(np.float32) for k in ("wq", "wk", "wv"))
    wo = np.ascontiguousarray(np.asarray(inputs["wo"]).astype(ml_dtypes.bfloat16))
    cos = np.asarray(inputs["freqs_cos"]).astype(np.float32)   # [S, 32]
    sin = np.asarray(inputs["freqs_sin"]).astype(np.float32)
    # xT layout [128 part, DT, S]: [p, t, s] = x[s, 128 t + p]  (shared)
    xTl = np.ascontiguousarray(
        x.T.reshape(DT, 128, SEQ).transpose(1, 0, 2).astype(ml_dtypes.bfloat16))
    # cosT/sinT [128, S]: row m = freq m%32, replicated x4  (shared)
    cosT = np.ascontiguousarray(
        np.tile(cos.T, (4, 1)).astype(ml_dtypes.bfloat16))
    sinT = np.ascontiguousarray(
        np.tile(sin.T, (4, 1)).astype(ml_dtypes.bfloat16))
    # triangle mask for the diagonal 128x128 block (keep col >= row)
    tri = (np.arange(128)[None, :] >= np.arange(128)[:, None]).astype(np.float32)
    tri2 = np.ascontiguousarray(
        np.broadcast_to(tri[:, None, :], (128, 2, 128)).astype(ml_dtypes.bfloat16))
    # tiled identity [I64 I64]: repk[r, c] = 1 iff c % 64 == r
    repk = np.ascontiguousarray(
        (np.arange(128)[None, :] % 64 == np.arange(64)[:, None])
        .astype(ml_dtypes.bfloat16))

    wq4 = wq.reshape(DIM, 32, 32, 2)       # [d_in, head, freq, r/i]
    wk4 = wk.reshape(DIM, 8, 32, 2)
    wv3 = wv.reshape(DIM, 8, HD)

    def lhsT_tiles(w2d):                   # [2048, 128] -> [128, DT, 128]
        return np.ascontiguousarray(
            w2d.reshape(DT, 128, 128).transpose(1, 0, 2)
            .astype(ml_dtypes.bfloat16))

    in_maps = []
    for c in range(NC_CORES):
        # qR cols m: head 4c + m//32, freq m%32, real part; qI imaginary
        wqR = wq4[:, 4 * c:4 * c + 4, :, 0].reshape(DIM, 128)
        wqI = wq4[:, 4 * c:4 * c + 4, :, 1].reshape(DIM, 128)
        # wkv cols: [v(64) | kr(32) | ki(32)] for kv-head c
        wkvc = np.concatenate([wv3[:, c, :], wk4[:, c, :, 0],
                               wk4[:, c, :, 1]], axis=1)
        in_maps.append({
            "xT": xTl,
            "wqR": lhsT_tiles(wqR),
            "wqI": lhsT_tiles(wqI),
            "wkv": lhsT_tiles(wkvc),
            "wo": wo,
            "cosT": cosT,
            "sinT": sinT,
            "tri2": tri2,
            "repk": repk,
        })
    return in_maps


def kernel(**inputs):
    from concourse.bass_utils import run_bass_kernel_spmd

    nc = _get_nc()
    in_maps = _shard(inputs)
    res = run_bass_kernel_spmd(nc, in_maps, core_ids=list(range(NC_CORES)))
    out = np.concatenate([res.results[c]["out"] for c in range(NC_CORES)], axis=0)
    return out[None].astype(np.float32)


# revision 41
# speedup vs baseline: 1.3174x; 1.3174x over previous
"""Tensor-parallel GQA attention forward for one TRN2 chip (8 NeuronCores).

Strategy (8-way tensor parallel over heads, no pre-attention collectives):
  - each core owns 4 q-heads + 1 kv-head and projects them DIRECTLY in the
    transposed layout from the full x (host supplies xT [128, 16, 2048]
    bf16, streamed in 512-column chunks): out[d, s] = w[:, d]^T @ xT.
    This removes the three pre-attention AllToAlls entirely - the first
    collective in the kernel (attnT redistribution) is not needed until
    attention pair-0 is done, so the multi-rank launch skew that gates the
    first collective costs nothing.
  - q is projected as separate real/imag row-blocks (qR rows = [h0r h1r
    h2r h3r], qI likewise, host-permuted weight columns), so RoPE runs as
    six 128-partition DVE multiplies against host-prepared cosT/sinT
    [freq, seq] tables; k rides the same scheme in a packed [kr ki v]
    projection
  - scores contract r- and i- halves in two accumulating K=32 matmuls per
    head (explicit tile_position row-tiling keeps head pairs concurrent);
    kR/kI are replicated x4 across partition blocks so lhsT/rhs bases line
    up; v is PE-transposed into [s, d] for the PV matmul
  - scores land transposed (S^T[k, q]) in PSUM so exp runs straight out of
    PSUM; softmax denominators come free as ones-columns in the PV matmul;
    causal masking = skipping k-tiles above the diagonal, a column
    trapezoid on the diagonal band, and a 128-wide triangle multiply
  - attention pair-0 interleaves with the tail of the projections and
    pair-1 interleaves with the even half of the output projection, so the
    PE stays dense (HAM stays at 2.4 GHz) through the ACT-paced softmax
  - an AllToAll flips head-sharded attnT to sequence-sharded; remaining
    even-half groups fill the final collective's window, then the odd half
    runs and the halves are summed
  - compute dtype bf16 (fp32 PSUM accumulation), output fp32
"""

import numpy as np

NC_CORES = 8
SEQ = 2048
DIM = 2048
HD = 64            # head dim
SC = SEQ // NC_CORES   # 256: sequence rows per core (output shard)
CH = 512           # q-chunk width for attention / projection s-chunk
NCH = SEQ // CH    # 4
KT = SEQ // 128    # 16 k-tiles
DT = DIM // 128    # 16 d-tiles

_CACHE = {}


def _build_nc():
    import concourse.bass as bass
    import concourse.mybir as mybir
    import concourse.tile as tile
    from concourse import bacc
    from concourse.masks import make_identity

    BF = mybir.dt.bfloat16
    F32 = mybir.dt.float32
    MUL = mybir.AluOpType.mult
    ADD = mybir.AluOpType.add
    SUB = mybir.AluOpType.subtract

    nc = bacc.Bacc("TRN2", target_bir_lowering=False, debug=False,
                   num_devices=NC_CORES)

    # ---- external I/O (per-core shards) ----
    xT = nc.dram_tensor("xT", [128, DT, SEQ], BF, kind="ExternalInput")
    wqR = nc.dram_tensor("wqR", [128, DT, 128], BF, kind="ExternalInput")
    wqI = nc.dram_tensor("wqI", [128, DT, 128], BF, kind="ExternalInput")
    wkv = nc.dram_tensor("wkv", [128, DT, 128], BF, kind="ExternalInput")
    wo = nc.dram_tensor("wo", [DIM, DIM], BF, kind="ExternalInput")
    cosT_in = nc.dram_tensor("cosT", [128, SEQ], BF, kind="ExternalInput")
    sinT_in = nc.dram_tensor("sinT", [128, SEQ], BF, kind="ExternalInput")
    repk_in = nc.dram_tensor("repk", [64, 128], BF, kind="ExternalInput")
    tri2_in = nc.dram_tensor("tri2", [128, 2, 128], BF, kind="ExternalInput")
    out = nc.dram_tensor("out", [SC, DIM], F32, kind="ExternalOutput")

    groups = [list(range(NC_CORES))]

    with tile.TileContext(nc) as tc:
        # DRAM bounce buffers for the output-side collectives
        a2a_in0, _ = tc.tile([NC_CORES, 128, SC], BF,
                             space=bass.MemorySpace.DRAM, name="a2a_in0")
        a2a_out0, _ = tc.tile([NC_CORES, 128, SC], BF,
                              space=bass.MemorySpace.DRAM,
                              addr_space="Shared", name="a2a_out0")
        a2a_in1, _ = tc.tile([NC_CORES, 128, SC], BF,
                             space=bass.MemorySpace.DRAM, name="a2a_in1")
        a2a_out1, _ = tc.tile([NC_CORES, 128, SC], BF,
                              space=bass.MemorySpace.DRAM,
                              addr_space="Shared", name="a2a_out1")

        with tc.tile_pool(name="persist", bufs=1) as pp, \
             tc.tile_pool(name="work", bufs=2) as wp, \
             tc.tile_pool(name="psum", bufs=2, space="PSUM") as psp:

            # weights (small: 1.5 MB total) and tables load first
            wq_sb = pp.tile([128, 2, DT, 128], BF, name="wq_sb")
            nc.sync.dma_start(wq_sb[:, 0, :, :], wqR[:])
            nc.sync.dma_start(wq_sb[:, 1, :, :], wqI[:])
            wkv_sb = pp.tile([128, DT, 128], BF, name="wkv_sb")
            nc.sync.dma_start(wkv_sb[:], wkv[:])
            cosT = pp.tile([128, SEQ], BF, name="cosT")
            sinT = pp.tile([128, SEQ], BF, name="sinT")
            nc.sync.dma_start(cosT[:], cosT_in[:])
            nc.sync.dma_start(sinT[:], sinT_in[:])
            tri2 = pp.tile([128, 2, 128], BF, name="tri2")
            nc.sync.dma_start(tri2[:], tri2_in[:])
            repk_sb = pp.tile([64, 128], BF, name="repk_sb")
            nc.sync.dma_start(repk_sb[:], repk_in[:])
            ident = pp.tile([128, 128], BF, name="ident")
            make_identity(nc, ident[:])

            # roped q per pair: rows = [h0r(32) h0i(32) h1r(32) h1i(32)]
            qP = [pp.tile([128, SEQ], BF, name=f"qP{p}") for p in range(2)]
            # roped k, [kr ki] duplicated: rows = [kr ki kr ki]
            kT2 = pp.tile([128, SEQ], BF, name="kT2")
            v_sb = pp.tile([128, KT, 2 * HD], BF, name="v_sb")
            nc.gpsimd.memset(v_sb[:, :, HD:2 * HD], 1.0)

            # ---------------- direct projections, one 512-seq chunk at a time
            # Each chunk is built as a list of small steps (~4 matmuls each)
            # so it can drip into the attention loop as PE filler.
            def chunk_steps(sc):
                ssl = slice(CH * sc, CH * sc + CH)
                state = {}
                steps = []

                def load():
                    xc = wp.tile([128, DT, CH], BF, tag="xring", bufs=2,
                                 name="xc")
                    e0 = nc.scalar if sc % 2 == 0 else nc.gpsimd
                    e1 = nc.gpsimd if sc % 2 == 0 else nc.scalar
                    e0.dma_start(xc[:, 0:4, :], xT[:, 0:4, ssl])
                    e1.dma_start(xc[:, 4:8, :], xT[:, 4:8, ssl])
                    e0.dma_start(xc[:, 8:12, :], xT[:, 8:12, ssl])
                    e1.dma_start(xc[:, 12:16, :], xT[:, 12:16, ssl])
                    state["xc"] = xc
                steps.append(load)

                # packed [v(64) | kr(32) | ki(32)] projection
                def kv_mms(i0):
                    def f():
                        if i0 == 0:
                            state["pkv"] = psp.tile([128, CH], F32, tag="ps",
                                                    bufs=3, name="pkv")
                        for dt in range(i0, i0 + 4):
                            nc.tensor.matmul(
                                state["pkv"][:], wkv_sb[:, dt, :],
                                state["xc"][:, dt, :],
                                start=(dt == 0), stop=(dt == DT - 1))
                    return f
                for i0 in range(0, DT, 4):
                    steps.append(kv_mms(i0))

                def kv_post():
                    kvraw = wp.tile([128, CH], BF, tag="kvraw", bufs=2,
                                    name="kvraw")
                    nc.scalar.copy(kvraw[:], state["pkv"][:])
                    state["kvraw"] = kvraw
                    # k RoPE on rows 64:128 (cosT's freq pattern repeats
                    # every 32 rows - read the rows matching each input's
                    # partitions; DVE TT needs equal input base partitions)
                    kro = wp.tile([64, CH], BF, tag="kro", bufs=2,
                                  name="kro")
                    kr, ki = kvraw[64:96, :], kvraw[96:128, :]
                    ta = wp.tile([32, 2, CH], BF, tag="kta", bufs=2, name="kta")
                    nc.vector.tensor_tensor(ta[:, 0, :], kr,
                                            cosT[64:96, ssl], MUL)
                    nc.vector.tensor_tensor(ta[:, 1, :], ki,
                                            sinT[96:128, ssl], MUL)
                    nc.vector.tensor_tensor(kro[0:32, :], ta[:, 0, :],
                                            ta[:, 1, :], SUB)
                    nc.vector.tensor_tensor(ta[:, 0, :], kr,
                                            sinT[64:96, ssl], MUL)
                    nc.vector.tensor_tensor(ta[:, 1, :], ki,
                                            cosT[96:128, ssl], MUL)
                    nc.vector.tensor_tensor(kro[32:64, :], ta[:, 0, :],
                                            ta[:, 1, :], ADD)
                    state["kro"] = kro
                steps.append(kv_post)

                # duplicate roped [kr ki] into rows [kr ki kr ki] via a
                # tiled-identity matmul (lane-correct partition broadcast)
                def k_rep():
                    prep = psp.tile([128, CH], F32, tag="ps", bufs=3,
                                    name="prep")
                    nc.tensor.matmul(prep[:], repk_sb[:], state["kro"][:],
                                     start=True, stop=True)
                    nc.vector.tensor_copy(kT2[:, ssl], prep[:])
                steps.append(k_rep)

                # v: PE-transpose [128, 128] blocks; keep the v rows
                def v_tr(i):
                    def f():
                        kt = 4 * sc + i
                        tv = psp.tile([128, 128], BF, tag="ps", bufs=3,
                                      name="tv")
                        nc.tensor.transpose(tv[:],
                                            state["kvraw"][:, 128 * i:128 * i + 128],
                                            ident[:])
                        nc.vector.tensor_copy(v_sb[:, kt, 0:HD],
                                              tv[0:128, 0:64])
                    return f
                for i in range(4):
                    steps.append(v_tr(i))

                # q r/i projections
                def q_mms(ri, i0):
                    def f():
                        if i0 == 0:
                            state[f"pq{ri}"] = psp.tile([128, CH], F32,
                                                        tag="ps", bufs=3,
                                                        name=f"pq{ri}")
                        for dt in range(i0, i0 + 4):
                            nc.tensor.matmul(
                                state[f"pq{ri}"][:], wq_sb[:, ri, dt, :],
                                state["xc"][:, dt, :],
                                start=(dt == 0), stop=(dt == DT - 1))
                    return f

                def q_post(ri):
                    def f():
                        qraw = wp.tile([128, CH], BF, tag=f"qraw{ri}", bufs=2,
                                       name=f"qraw{ri}")
                        nc.scalar.copy(qraw[:], state[f"pq{ri}"][:])
                        state[f"qraw{ri}"] = qraw
                    return f
                for ri in range(2):
                    for i0 in range(0, DT, 4):
                        steps.append(q_mms(ri, i0))
                    steps.append(q_post(ri))

                def q_rope():
                    # products on all 128 rows at once, then per-head 32-row
                    # writes into the pair tiles' [hr | hi] row layout
                    ct4, st4 = cosT[:, ssl], sinT[:, ssl]
                    qr, qi = state["qraw0"][:], state["qraw1"][:]
                    tq = wp.tile([128, 2, CH], BF, tag="qta", bufs=2, name="tq")
                    nc.vector.tensor_tensor(tq[:, 0, :], qr, ct4, MUL)
                    nc.vector.tensor_tensor(tq[:, 1, :], qi, st4, MUL)
                    for g in range(4):
                        rs = slice(32 * g, 32 * g + 32)
                        dst = qP[g // 2][64 * (g % 2):64 * (g % 2) + 32, ssl]
                        nc.vector.tensor_tensor(dst, tq[rs, 0, :],
                                                tq[rs, 1, :], SUB)
                    nc.vector.tensor_tensor(tq[:, 0, :], qr, st4, MUL)
                    nc.vector.tensor_tensor(tq[:, 1, :], qi, ct4, MUL)
                    for g in range(4):
                        rs = slice(32 * g, 32 * g + 32)
                        dst = qP[g // 2][64 * (g % 2) + 32:64 * (g % 2) + 64,
                                         ssl]
                        nc.vector.tensor_tensor(dst, tq[rs, 0, :],
                                                tq[rs, 1, :], ADD)
                steps.append(q_rope)
                return steps

            drip = {"steps": []}

            def drip_run(n):
                for _ in range(n):
                    if not drip["steps"]:
                        return
                    drip["steps"].pop(0)()

            def drip_flush():
                drip_run(len(drip["steps"]) + 1)

            # ---------------- attention ----------------
            attnT = pp.tile([128, 2, SEQ], BF, name="attnT")

            def attention(pair, j, interleave=None):
                nkt = 4 * j + 4
                nfull = nkt - 4
                pso0 = psp.tile([2 * HD, CH], F32, tag="ps", bufs=3, name="pso0")
                pso1 = psp.tile([2 * HD, CH], F32, tag="ps", bufs=3, name="pso1")
                qsl = slice(CH * j, CH * j + CH)
                qPt = qP[pair]

                def pv(ep_h0, ep_h1, kt, c0):
                    nc.tensor.matmul(pso0[:, c0:CH], v_sb[:, kt, :], ep_h0,
                                     start=(kt == 0), stop=(kt == nkt - 1))
                    nc.tensor.matmul(pso1[:, c0:CH], v_sb[:, kt, :], ep_h1,
                                     start=(kt == 0), stop=(kt == nkt - 1))

                # software pipeline: PV of k-tile i runs one iteration behind
                # its exp, with the drip filler issued in between on the PE
                prev = None
                for kt in range(nkt):
                    t = kt - nfull            # >= 0 on the diagonal band
                    c0 = 128 * t if t >= 0 else 0
                    ks = slice(128 * kt, 128 * kt + 128)
                    qs = slice(CH * j + c0, CH * j + CH)
                    sp = psp.tile([128, 2, CH], F32, tag="sp2", bufs=2,
                                  name="sp")
                    for h in range(2):
                        nc.tensor.matmul(sp[:, h, c0:CH],
                                         kT2[64 * h:64 * h + 64, ks],
                                         qPt[64 * h:64 * h + 64, qs],
                                         start=True, stop=True)
                    ep = wp.tile([128, 2, CH], BF, tag="exps", bufs=4,
                                 name="ep")
                    nc.scalar.activation(ep[:, :, c0:CH], sp[:, :, c0:CH],
                                         mybir.ActivationFunctionType.Exp,
                                         scale=0.125)
                    if t >= 0:
                        nc.vector.tensor_tensor(ep[:, :, c0:c0 + 128],
                                                ep[:, :, c0:c0 + 128],
                                                tri2[:], MUL)
                    if interleave is not None:
                        interleave(j, kt)
                    if prev is not None:
                        pv(*prev)
                    prev = (ep[:, 0, c0:CH], ep[:, 1, c0:CH], kt, c0)
                pv(*prev)

                for h, pso in ((0, pso0), (1, pso1)):
                    bc = wp.tile([64, CH], F32, tag="bcast", bufs=2, name="bc")
                    nc.vector.tensor_copy(bc[:], pso[HD:2 * HD, :])
                    rc = wp.tile([64, CH], F32, tag="rcp", bufs=2, name="rc")
                    nc.vector.reciprocal_approx_fast(out=rc[:], in_=bc[:])
                    nc.vector.tensor_tensor(
                        attnT[64 * h:64 * h + 64, pair, qsl],
                        pso[0:HD, :], rc[:], MUL)

            # ---------------- output projection helpers ----------------
            woA = pp.tile([128, DT // 2, DIM], BF, name="woA")
            woB = pp.tile([128, DT // 2, DIM], BF, name="woB")
            a2a_sb0 = pp.tile([128, NC_CORES, SC], BF, name="a2a_sb0")
            a2a_sb1 = pp.tile([128, NC_CORES, SC], BF, name="a2a_sb1")
            partials = pp.tile([128, 2 * NCH, CH], BF, tag="proj",
                               name="partials")
            evens = [2 * src for src in range(NC_CORES)]
            odds = [2 * src + 1 for src in range(NC_CORES)]
            chunks = [(qt, nch) for qt in range(2) for nch in range(NCH)]

            def op_mm(psf, qt, nsl, g, start, stop):
                w_ap = (woA[:, g, nsl] if g < DT // 2
                        else woB[:, g - DT // 2, nsl])
                a_ap = (a2a_sb0[:, g // 2, 128 * qt:128 * qt + 128] if g % 2 == 0
                        else a2a_sb1[:, g // 2, 128 * qt:128 * qt + 128])
                nc.tensor.matmul(psf[:], a_ap, w_ap, start=start, stop=stop)

            ev_state = {"psf": None, "n": 0}

            def even_steps(nsteps):
                for _ in range(nsteps):
                    n = ev_state["n"]
                    if n >= 64:
                        return
                    i8, i = divmod(n, NC_CORES)
                    qt, nch2 = chunks[i8]
                    if i == 0:
                        ev_state["psf"] = psp.tile([128, CH], F32, tag="psf",
                                                   bufs=1, name="psfE")
                    nsl = slice(CH * nch2, CH * nch2 + CH)
                    op_mm(ev_state["psf"], qt, nsl, evens[i],
                          i == 0, i == NC_CORES - 1)
                    if i == NC_CORES - 1:
                        nc.vector.tensor_copy(partials[:, i8, :],
                                              ev_state["psf"][:])
                    ev_state["n"] = n + 1

            # ---------------- pair-0 attention, proj-interleaved ----------
            # proj chunk c+1 drips into attention chunk c as PE filler and
            # is flushed before the attention chunk that first needs it
            for f in chunk_steps(0):
                f()
            drip_rate = (5, 3, 2, 0)

            def inter0(jj, kt):
                drip_run(drip_rate[jj])

            for j in range(NCH):
                if j < NCH - 1:
                    drip["steps"] = chunk_steps(j + 1)
                attention(0, j, interleave=inter0)
                drip_flush()
                nc.sync.dma_start(
                    a2a_in0[2 * j:2 * j + 2, :, :]
                    .rearrange("d p m -> p d m"),
                    attnT[:, 0, CH * j:CH * j + CH]
                    .rearrange("p (d m) -> p d m", m=SC))
                # anchored wo prefetch (the scheduler hoists dep-free DMAs)
                nc.vector.tensor_copy(woA[0:1, 2 * j, 0:1],
                                      attnT[0:1, 0, CH * j:CH * j + 1])
                nc.sync.dma_start(
                    woA[:, 2 * j:2 * j + 2, :],
                    wo[256 * j:256 * j + 256, :].rearrange("(t p) n -> p t n",
                                                           p=128))
                if j >= 2:   # woB too: needed by the interleaved even groups
                    jb = j - 2
                    nc.vector.tensor_copy(woB[0:1, 4 * jb, 0:1],
                                          attnT[0:1, 0, CH * j:CH * j + 1])
                    nc.gpsimd.dma_start(
                        woB[:, 4 * jb:4 * jb + 4, :],
                        wo[1024 + 512 * jb:1024 + 512 * jb + 512, :]
                        .rearrange("(t p) n -> p t n", p=128))
            nc.gpsimd.collective_compute(
                "AllToAll", mybir.AluOpType.bypass,
                replica_groups=groups, ins=[a2a_in0.opt()], outs=[a2a_out0.opt()],
            )
            for half in range(2):
                nc.sync.dma_start(
                    a2a_sb0[:, :, 128 * half:128 * half + 128],
                    a2a_out0[:, :, 128 * half:128 * half + 128]
                    .rearrange("s p m -> p s m"))

            # ---------------- pair-1 attention (pure) ----------------
            # Chunk order 1,0,2,3: chunk 0 is all-diagonal (tri-mult-gated on
            # the DVE) and would stall right behind pair-0's normalize chain.
            # No interleave: pair-1 finishes (and the final A2A triggers) as
            # early as possible; the even outproj then fills the A2A window.
            p1_order = (1, 0, 2, 3)
            for j in p1_order:
                attention(1, j)
                nc.sync.dma_start(
                    a2a_in1[2 * j:2 * j + 2, :, :]
                    .rearrange("d p m -> p d m"),
                    attnT[:, 1, CH * j:CH * j + CH]
                    .rearrange("p (d m) -> p d m", m=SC))

            # ---------------- final A2A + remaining outproj ----------------
            # evens are emitted BEFORE the collective call so tile's block
            # ordering doesn't gate them behind it - they only depend on
            # a2a_sb0, so they execute inside the collective's skew + wire
            # window on the PE
            even_steps(64)
            nc.gpsimd.collective_compute(
                "AllToAll", mybir.AluOpType.bypass,
                replica_groups=groups, ins=[a2a_in1.opt()], outs=[a2a_out1.opt()],
            )
            for half, eng in ((0, nc.sync), (1, nc.gpsimd)):
                eng.dma_start(
                    a2a_sb1[:, :, 128 * half:128 * half + 128],
                    a2a_out1[:, :, 128 * half:128 * half + 128]
                    .rearrange("s p m -> p s m"))

            store_engs = (nc.sync, nc.scalar, nc.gpsimd)
            for i8, (qt, nch2) in enumerate(chunks):
                psf = psp.tile([128, CH], F32, tag="ps", bufs=3, name="psfO")
                nsl = slice(CH * nch2, CH * nch2 + CH)
                for i, g in enumerate(odds):
                    op_mm(psf, qt, nsl, g, i == 0, i == NC_CORES - 1)
                osb = wp.tile([128, CH], F32, tag="osb", bufs=2, name="osb")
                nc.vector.tensor_tensor(osb[:], psf[:], partials[:, i8, :], ADD)
                store_engs[i8 % 3].dma_start(out[128 * qt:128 * qt + 128, nsl],
                                             osb[:])

    nc.finalize()
    return nc


def _get_nc():
    if "nc" not in _CACHE:
        _CACHE["nc"] = _build_nc()
    return _CACHE["nc"]


def _shard(inputs):
    import ml_dtypes
    x = np.ascontiguousarray(inputs["x"][0].astype(np.float32))          # [S, D]
    wq, wk, wv = (np.asarray(inputs[k]).astype(np.float32) for k in ("wq", "wk", "wv"))
    wo = np.ascontiguousarray(np.asarray(inputs["wo"]).astype(ml_dtypes.bfloat16))
    cos = np.asarray(inputs["freqs_cos"]).astype(np.float32)   # [S, 32]
    sin = np.asarray(inputs["freqs_sin"]).astype(np.float32)
    # xT layout [128 part, DT, S]: [p, t, s] = x[s, 128 t + p]  (shared)
    xTl = np.ascontiguousarray(
        x.T.reshape(DT, 128, SEQ).transpose(1, 0, 2).astype(ml_dtypes.bfloat16))
    # cosT/sinT [128, S]: row m = freq m%32, replicated x4  (shared)
    cosT = np.ascontiguousarray(
        np.tile(cos.T, (4, 1)).astype(ml_dtypes.bfloat16))
    sinT = np.ascontiguousarray(
        np.tile(sin.T, (4, 1)).astype(ml_dtypes.bfloat16))
    # triangle mask for the diagonal 128x128 block (keep col >= row)
    tri = (np.arange(128)[None, :] >= np.arange(128)[:, None]).astype(np.float32)
    tri2 = np.ascontiguousarray(
        np.broadcast_to(tri[:, None, :], (128, 2, 128)).astype(ml_dtypes.bfloat16))
    # tiled identity [I64 I64]: repk[r, c] = 1 iff c % 64 == r
    repk = np.ascontiguousarray(
        (np.arange(128)[None, :] % 64 == np.arange(64)[:, None])
        .astype(ml_dtypes.bfloat16))

    wq4 = wq.reshape(DIM, 32, 32, 2)       # [d_in, head, freq, r/i]
    wk4 = wk.reshape(DIM, 8, 32, 2)
    wv3 = wv.reshape(DIM, 8, HD)

    def lhsT_tiles(w2d):                   # [2048, 128] -> [128, DT, 128]
        return np.ascontiguousarray(
            w2d.reshape(DT, 128, 128).transpose(1, 0, 2)
            .astype(ml_dtypes.bfloat16))

    in_maps = []
    for c in range(NC_CORES):
        # qR cols m: head 4c + m//32, freq m%32, real part; qI imaginary
        wqR = wq4[:, 4 * c:4 * c + 4, :, 0].reshape(DIM, 128)
        wqI = wq4[:, 4 * c:4 * c + 4, :, 1].reshape(DIM, 128)
        # wkv cols: [v(64) | kr(32) | ki(32)] for kv-head c
        wkvc = np.concatenate([wv3[:, c, :], wk4[:, c, :, 0],
                               wk4[:, c, :, 1]], axis=1)
        in_maps.append({
            "xT": xTl,
            "wqR": lhsT_tiles(wqR),
            "wqI": lhsT_tiles(wqI),
            "wkv": lhsT_tiles(wkvc),
            "wo": wo,
            "cosT": cosT,
            "sinT": sinT,
            "tri2": tri2,
            "repk": repk,
        })
    return in_maps


def kernel(**inputs):
    from concourse.bass_utils import run_bass_kernel_spmd

    nc = _get_nc()
    in_maps = _shard(inputs)
    res = run_bass_kernel_spmd(nc, in_maps, core_ids=list(range(NC_CORES)))
    out = np.concatenate([res.results[c]["out"] for c in range(NC_CORES)], axis=0)
    return out[None].astype(np.float32)


# revision 42
# speedup vs baseline: 1.3306x; 1.0100x over previous
"""Tensor-parallel GQA attention forward for one TRN2 chip (8 NeuronCores).

Strategy (8-way tensor parallel over heads, no pre-attention collectives):
  - each core owns 4 q-heads + 1 kv-head and projects them DIRECTLY in the
    transposed layout from the full x (host supplies xT [128, 16, 2048]
    bf16, streamed in 512-column chunks): out[d, s] = w[:, d]^T @ xT.
    This removes the three pre-attention AllToAlls entirely - the first
    collective in the kernel (attnT redistribution) is not needed until
    attention pair-0 is done, so the multi-rank launch skew that gates the
    first collective costs nothing.
  - q is projected as separate real/imag row-blocks (qR rows = [h0r h1r
    h2r h3r], qI likewise, host-permuted weight columns), so RoPE runs as
    six 128-partition DVE multiplies against host-prepared cosT/sinT
    [freq, seq] tables; k rides the same scheme in a packed [kr ki v]
    projection
  - scores contract r- and i- halves in two accumulating K=32 matmuls per
    head (explicit tile_position row-tiling keeps head pairs concurrent);
    kR/kI are replicated x4 across partition blocks so lhsT/rhs bases line
    up; v is PE-transposed into [s, d] for the PV matmul
  - scores land transposed (S^T[k, q]) in PSUM so exp runs straight out of
    PSUM; softmax denominators come free as ones-columns in the PV matmul;
    causal masking = skipping k-tiles above the diagonal, a column
    trapezoid on the diagonal band, and a 128-wide triangle multiply
  - attention pair-0 interleaves with the tail of the projections and
    pair-1 interleaves with the even half of the output projection, so the
    PE stays dense (HAM stays at 2.4 GHz) through the ACT-paced softmax
  - an AllToAll flips head-sharded attnT to sequence-sharded; remaining
    even-half groups fill the final collective's window, then the odd half
    runs and the halves are summed
  - compute dtype bf16 (fp32 PSUM accumulation), output fp32
"""

import numpy as np

NC_CORES = 8
SEQ = 2048
DIM = 2048
HD = 64            # head dim
SC = SEQ // NC_CORES   # 256: sequence rows per core (output shard)
CH = 512           # q-chunk width for attention / projection s-chunk
NCH = SEQ // CH    # 4
KT = SEQ // 128    # 16 k-tiles
DT = DIM // 128    # 16 d-tiles

_CACHE = {}


def _build_nc():
    import concourse.bass as bass
    import concourse.mybir as mybir
    import concourse.tile as tile
    from concourse import bacc
    from concourse.masks import make_identity

    BF = mybir.dt.bfloat16
    F32 = mybir.dt.float32
    MUL = mybir.AluOpType.mult
    ADD = mybir.AluOpType.add
    SUB = mybir.AluOpType.subtract

    nc = bacc.Bacc("TRN2", target_bir_lowering=False, debug=False,
                   num_devices=NC_CORES)

    # ---- external I/O (per-core shards) ----
    xT = nc.dram_tensor("xT", [128, DT, SEQ], BF, kind="ExternalInput")
    wqR = nc.dram_tensor("wqR", [128, DT, 128], BF, kind="ExternalInput")
    wqI = nc.dram_tensor("wqI", [128, DT, 128], BF, kind="ExternalInput")
    wkv = nc.dram_tensor("wkv", [128, DT, 128], BF, kind="ExternalInput")
    wo = nc.dram_tensor("wo", [DIM, DIM], BF, kind="ExternalInput")
    cosT_in = nc.dram_tensor("cosT", [128, SEQ], BF, kind="ExternalInput")
    sinT_in = nc.dram_tensor("sinT", [128, SEQ], BF, kind="ExternalInput")
    repk_in = nc.dram_tensor("repk", [64, 128], BF, kind="ExternalInput")
    tri2_in = nc.dram_tensor("tri2", [128, 2, 128], BF, kind="ExternalInput")
    out = nc.dram_tensor("out", [SC, DIM], F32, kind="ExternalOutput")

    groups = [list(range(NC_CORES))]

    with tile.TileContext(nc) as tc:
        # DRAM bounce buffers for the output-side collectives
        a2a_in0, _ = tc.tile([NC_CORES, 128, SC], BF,
                             space=bass.MemorySpace.DRAM, name="a2a_in0")
        a2a_out0, _ = tc.tile([NC_CORES, 128, SC], BF,
                              space=bass.MemorySpace.DRAM,
                              addr_space="Shared", name="a2a_out0")
        a2a_in1, _ = tc.tile([NC_CORES, 128, SC], BF,
                             space=bass.MemorySpace.DRAM, name="a2a_in1")
        a2a_out1, _ = tc.tile([NC_CORES, 128, SC], BF,
                              space=bass.MemorySpace.DRAM,
                              addr_space="Shared", name="a2a_out1")

        with tc.tile_pool(name="persist", bufs=1) as pp, \
             tc.tile_pool(name="work", bufs=2) as wp, \
             tc.tile_pool(name="psum", bufs=2, space="PSUM") as psp:

            # weights (small: 1.5 MB total) and tables load first
            wq_sb = pp.tile([128, 2, DT, 128], BF, name="wq_sb")
            nc.sync.dma_start(wq_sb[:, 0, :, :], wqR[:])
            nc.sync.dma_start(wq_sb[:, 1, :, :], wqI[:])
            wkv_sb = pp.tile([128, DT, 128], BF, name="wkv_sb")
            nc.sync.dma_start(wkv_sb[:], wkv[:])
            cosT = pp.tile([128, SEQ], BF, name="cosT")
            sinT = pp.tile([128, SEQ], BF, name="sinT")
            nc.sync.dma_start(cosT[:], cosT_in[:])
            nc.sync.dma_start(sinT[:], sinT_in[:])
            tri2 = pp.tile([128, 2, 128], BF, name="tri2")
            nc.sync.dma_start(tri2[:], tri2_in[:])
            repk_sb = pp.tile([64, 128], BF, name="repk_sb")
            nc.sync.dma_start(repk_sb[:], repk_in[:])
            ident = pp.tile([128, 128], BF, name="ident")
            make_identity(nc, ident[:])

            # roped q per pair: rows = [h0r(32) h0i(32) h1r(32) h1i(32)]
            qP = [pp.tile([128, SEQ], BF, name=f"qP{p}") for p in range(2)]
            # roped k, [kr ki] duplicated: rows = [kr ki kr ki]
            kT2 = pp.tile([128, SEQ], BF, name="kT2")
            v_sb = pp.tile([128, KT, 2 * HD], BF, name="v_sb")
            nc.gpsimd.memset(v_sb[:, :, HD:2 * HD], 1.0)

            # ---------------- direct projections, one 512-seq chunk at a time
            # Each chunk is built as a list of small steps (~4 matmuls each)
            # so it can drip into the attention loop as PE filler.
            def chunk_steps(sc):
                ssl = slice(CH * sc, CH * sc + CH)
                state = {}
                steps = []

                def load():
                    xc = wp.tile([128, DT, CH], BF, tag="xring", bufs=2,
                                 name="xc")
                    e0 = nc.scalar if sc % 2 == 0 else nc.gpsimd
                    e1 = nc.gpsimd if sc % 2 == 0 else nc.scalar
                    e0.dma_start(xc[:, 0:4, :], xT[:, 0:4, ssl])
                    e1.dma_start(xc[:, 4:8, :], xT[:, 4:8, ssl])
                    e0.dma_start(xc[:, 8:12, :], xT[:, 8:12, ssl])
                    e1.dma_start(xc[:, 12:16, :], xT[:, 12:16, ssl])
                    state["xc"] = xc
                steps.append(load)

                # packed [v(64) | kr(32) | ki(32)] projection
                def kv_mms(i0):
                    def f():
                        if i0 == 0:
                            state["pkv"] = psp.tile([128, CH], F32, tag="ps",
                                                    bufs=3, name="pkv")
                        for dt in range(i0, i0 + 4):
                            nc.tensor.matmul(
                                state["pkv"][:], wkv_sb[:, dt, :],
                                state["xc"][:, dt, :],
                                start=(dt == 0), stop=(dt == DT - 1))
                    return f
                for i0 in range(0, DT, 4):
                    steps.append(kv_mms(i0))

                def kv_post():
                    kvraw = wp.tile([128, CH], BF, tag="kvraw", bufs=2,
                                    name="kvraw")
                    nc.scalar.copy(kvraw[:], state["pkv"][:])
                    state["kvraw"] = kvraw
                    # k RoPE on rows 64:128 (cosT's freq pattern repeats
                    # every 32 rows - read the rows matching each input's
                    # partitions; DVE TT needs equal input base partitions)
                    kro = wp.tile([64, CH], BF, tag="kro", bufs=2,
                                  name="kro")
                    kr, ki = kvraw[64:96, :], kvraw[96:128, :]
                    ta = wp.tile([32, 2, CH], BF, tag="kta", bufs=2, name="kta")
                    nc.vector.tensor_tensor(ta[:, 0, :], kr,
                                            cosT[64:96, ssl], MUL)
                    nc.vector.tensor_tensor(ta[:, 1, :], ki,
                                            sinT[96:128, ssl], MUL)
                    nc.vector.tensor_tensor(kro[0:32, :], ta[:, 0, :],
                                            ta[:, 1, :], SUB)
                    nc.vector.tensor_tensor(ta[:, 0, :], kr,
                                            sinT[64:96, ssl], MUL)
                    nc.vector.tensor_tensor(ta[:, 1, :], ki,
                                            cosT[96:128, ssl], MUL)
                    nc.vector.tensor_tensor(kro[32:64, :], ta[:, 0, :],
                                            ta[:, 1, :], ADD)
                    state["kro"] = kro
                steps.append(kv_post)

                # duplicate roped [kr ki] into rows [kr ki kr ki] via a
                # tiled-identity matmul (lane-correct partition broadcast)
                def k_rep():
                    prep = psp.tile([128, CH], F32, tag="ps", bufs=3,
                                    name="prep")
                    nc.tensor.matmul(prep[:], repk_sb[:], state["kro"][:],
                                     start=True, stop=True)
                    nc.vector.tensor_copy(kT2[:, ssl], prep[:])
                steps.append(k_rep)

                # v: PE-transpose [128, 128] blocks; keep the v rows
                def v_tr(i):
                    def f():
                        kt = 4 * sc + i
                        tv = psp.tile([128, 128], BF, tag="ps", bufs=3,
                                      name="tv")
                        nc.tensor.transpose(tv[:],
                                            state["kvraw"][:, 128 * i:128 * i + 128],
                                            ident[:])
                        nc.vector.tensor_copy(v_sb[:, kt, 0:HD],
                                              tv[0:128, 0:64])
                    return f
                for i in range(4):
                    steps.append(v_tr(i))

                # q r/i projections
                def q_mms(ri, i0):
                    def f():
                        if i0 == 0:
                            state[f"pq{ri}"] = psp.tile([128, CH], F32,
                                                        tag="ps", bufs=3,
                                                        name=f"pq{ri}")
                        for dt in range(i0, i0 + 4):
                            nc.tensor.matmul(
                                state[f"pq{ri}"][:], wq_sb[:, ri, dt, :],
                                state["xc"][:, dt, :],
                                start=(dt == 0), stop=(dt == DT - 1))
                    return f

                def q_post(ri):
                    def f():
                        qraw = wp.tile([128, CH], BF, tag=f"qraw{ri}", bufs=2,
                                       name=f"qraw{ri}")
                        nc.scalar.copy(qraw[:], state[f"pq{ri}"][:])
                        state[f"qraw{ri}"] = qraw
                    return f
                for ri in range(2):
                    for i0 in range(0, DT, 4):
                        steps.append(q_mms(ri, i0))
                    steps.append(q_post(ri))

                def q_rope():
                    # products on all 128 rows at once, then per-head 32-row
                    # writes into the pair tiles' [hr | hi] row layout
                    ct4, st4 = cosT[:, ssl], sinT[:, ssl]
                    qr, qi = state["qraw0"][:], state["qraw1"][:]
                    tq = wp.tile([128, 2, CH], BF, tag="qta", bufs=2, name="tq")
                    nc.vector.tensor_tensor(tq[:, 0, :], qr, ct4, MUL)
                    nc.vector.tensor_tensor(tq[:, 1, :], qi, st4, MUL)
                    for g in range(4):
                        rs = slice(32 * g, 32 * g + 32)
                        dst = qP[g // 2][64 * (g % 2):64 * (g % 2) + 32, ssl]
                        nc.vector.tensor_tensor(dst, tq[rs, 0, :],
                                                tq[rs, 1, :], SUB)
                    nc.vector.tensor_tensor(tq[:, 0, :], qr, st4, MUL)
                    nc.vector.tensor_tensor(tq[:, 1, :], qi, ct4, MUL)
                    for g in range(4):
                        rs = slice(32 * g, 32 * g + 32)
                        dst = qP[g // 2][64 * (g % 2) + 32:64 * (g % 2) + 64,
                                         ssl]
                        nc.vector.tensor_tensor(dst, tq[rs, 0, :],
                                                tq[rs, 1, :], ADD)
                steps.append(q_rope)
                return steps

            drip = {"steps": []}

            def drip_run(n):
                for _ in range(n):
                    if not drip["steps"]:
                        return
                    drip["steps"].pop(0)()

            def drip_flush():
                drip_run(len(drip["steps"]) + 1)

            # ---------------- attention ----------------
            attnT = pp.tile([128, 2, SEQ], BF, name="attnT")

            def attention(pair, j, interleave=None):
                nkt = 4 * j + 4
                nfull = nkt - 4
                pso0 = psp.tile([2 * HD, CH], F32, tag="ps", bufs=3, name="pso0")
                pso1 = psp.tile([2 * HD, CH], F32, tag="ps", bufs=3, name="pso1")
                qsl = slice(CH * j, CH * j + CH)
                qPt = qP[pair]

                def pv(ep_h0, ep_h1, kt, c0):
                    nc.tensor.matmul(pso0[:, c0:CH], v_sb[:, kt, :], ep_h0,
                                     start=(kt == 0), stop=(kt == nkt - 1))
                    nc.tensor.matmul(pso1[:, c0:CH], v_sb[:, kt, :], ep_h1,
                                     start=(kt == 0), stop=(kt == nkt - 1))

                # software pipeline: PV of k-tile i runs one iteration behind
                # its exp, with the drip filler issued in between on the PE
                prev = None
                for kt in range(nkt):
                    t = kt - nfull            # >= 0 on the diagonal band
                    c0 = 128 * t if t >= 0 else 0
                    ks = slice(128 * kt, 128 * kt + 128)
                    qs = slice(CH * j + c0, CH * j + CH)
                    sp = psp.tile([128, 2, CH], F32, tag="sp2", bufs=2,
                                  name="sp")
                    for h in range(2):
                        nc.tensor.matmul(sp[:, h, c0:CH],
                                         kT2[64 * h:64 * h + 64, ks],
                                         qPt[64 * h:64 * h + 64, qs],
                                         start=True, stop=True)
                    ep = wp.tile([128, 2, CH], BF, tag="exps", bufs=4,
                                 name="ep")
                    nc.scalar.activation(ep[:, :, c0:CH], sp[:, :, c0:CH],
                                         mybir.ActivationFunctionType.Exp,
                                         scale=0.125)
                    if t >= 0:
                        nc.vector.tensor_tensor(ep[:, :, c0:c0 + 128],
                                                ep[:, :, c0:c0 + 128],
                                                tri2[:], MUL)
                    if interleave is not None:
                        interleave(j, kt)
                    if prev is not None:
                        pv(*prev)
                    prev = (ep[:, 0, c0:CH], ep[:, 1, c0:CH], kt, c0)
                pv(*prev)

                for h, pso in ((0, pso0), (1, pso1)):
                    bc = wp.tile([64, CH], F32, tag="bcast", bufs=2, name="bc")
                    nc.vector.tensor_copy(bc[:], pso[HD:2 * HD, :])
                    rc = wp.tile([64, CH], F32, tag="rcp", bufs=2, name="rc")
                    nc.vector.reciprocal_approx_fast(out=rc[:], in_=bc[:])
                    nc.vector.tensor_tensor(
                        attnT[64 * h:64 * h + 64, pair, qsl],
                        pso[0:HD, :], rc[:], MUL)

            # ---------------- output projection helpers ----------------
            woA = pp.tile([128, DT // 2, DIM], BF, name="woA")
            woB = pp.tile([128, DT // 2, DIM], BF, name="woB")
            a2a_sb0 = pp.tile([128, NC_CORES, SC], BF, name="a2a_sb0")
            a2a_sb1 = pp.tile([128, NC_CORES, SC], BF, name="a2a_sb1")
            partials = pp.tile([128, 2 * NCH, CH], BF, tag="proj",
                               name="partials")
            evens = [2 * src for src in range(NC_CORES)]
            odds = [2 * src + 1 for src in range(NC_CORES)]
            chunks = [(qt, nch) for qt in range(2) for nch in range(NCH)]

            def op_mm(psf, qt, nsl, g, start, stop):
                w_ap = (woA[:, g, nsl] if g < DT // 2
                        else woB[:, g - DT // 2, nsl])
                a_ap = (a2a_sb0[:, g // 2, 128 * qt:128 * qt + 128] if g % 2 == 0
                        else a2a_sb1[:, g // 2, 128 * qt:128 * qt + 128])
                nc.tensor.matmul(psf[:], a_ap, w_ap, start=start, stop=stop)

            ev_state = {"psf": None, "n": 0}

            def even_steps(nsteps):
                for _ in range(nsteps):
                    n = ev_state["n"]
                    if n >= 64:
                        return
                    i8, i = divmod(n, NC_CORES)
                    qt, nch2 = chunks[i8]
                    if i == 0:
                        ev_state["psf"] = psp.tile([128, CH], F32, tag="psf",
                                                   bufs=1, name="psfE")
                    nsl = slice(CH * nch2, CH * nch2 + CH)
                    op_mm(ev_state["psf"], qt, nsl, evens[i],
                          i == 0, i == NC_CORES - 1)
                    if i == NC_CORES - 1:
                        nc.vector.tensor_copy(partials[:, i8, :],
                                              ev_state["psf"][:])
                    ev_state["n"] = n + 1

            # ---------------- pair-0 attention, proj-interleaved ----------
            # proj chunk c+1 drips into attention chunk c as PE filler and
            # is flushed before the attention chunk that first needs it
            for f in chunk_steps(0):
                f()
            drip_rate = (5, 3, 2, 0)

            def inter0(jj, kt):
                drip_run(drip_rate[jj])

            for j in range(NCH):
                if j < NCH - 1:
                    drip["steps"] = chunk_steps(j + 1)
                attention(0, j, interleave=inter0)
                drip_flush()
                nc.scalar.dma_start(
                    a2a_in0[2 * j:2 * j + 2, :, :]
                    .rearrange("d p m -> p d m"),
                    attnT[:, 0, CH * j:CH * j + CH]
                    .rearrange("p (d m) -> p d m", m=SC))
                # anchored wo prefetch (the scheduler hoists dep-free DMAs)
                nc.vector.tensor_copy(woA[0:1, 2 * j, 0:1],
                                      attnT[0:1, 0, CH * j:CH * j + 1])
                nc.sync.dma_start(
                    woA[:, 2 * j:2 * j + 2, :],
                    wo[256 * j:256 * j + 256, :].rearrange("(t p) n -> p t n",
                                                           p=128))
                if j >= 2:   # woB too: needed by the interleaved even groups
                    jb = j - 2
                    nc.vector.tensor_copy(woB[0:1, 4 * jb, 0:1],
                                          attnT[0:1, 0, CH * j:CH * j + 1])
                    nc.gpsimd.dma_start(
                        woB[:, 4 * jb:4 * jb + 4, :],
                        wo[1024 + 512 * jb:1024 + 512 * jb + 512, :]
                        .rearrange("(t p) n -> p t n", p=128))
            nc.gpsimd.collective_compute(
                "AllToAll", mybir.AluOpType.bypass,
                replica_groups=groups, ins=[a2a_in0.opt()], outs=[a2a_out0.opt()],
            )
            for half in range(2):
                nc.sync.dma_start(
                    a2a_sb0[:, :, 128 * half:128 * half + 128],
                    a2a_out0[:, :, 128 * half:128 * half + 128]
                    .rearrange("s p m -> p s m"))

            # ---------------- pair-1 attention (pure) ----------------
            # Chunk order 1,0,2,3: chunk 0 is all-diagonal (tri-mult-gated on
            # the DVE) and would stall right behind pair-0's normalize chain.
            # No interleave: pair-1 finishes (and the final A2A triggers) as
            # early as possible; the even outproj then fills the A2A window.
            p1_order = (1, 0, 2, 3)
            for j in p1_order:
                attention(1, j)
                nc.scalar.dma_start(
                    a2a_in1[2 * j:2 * j + 2, :, :]
                    .rearrange("d p m -> p d m"),
                    attnT[:, 1, CH * j:CH * j + CH]
                    .rearrange("p (d m) -> p d m", m=SC))

            # ---------------- final A2A + remaining outproj ----------------
            # evens are emitted BEFORE the collective call so tile's block
            # ordering doesn't gate them behind it - they only depend on
            # a2a_sb0, so they execute inside the collective's skew + wire
            # window on the PE
            even_steps(64)
            nc.gpsimd.collective_compute(
                "AllToAll", mybir.AluOpType.bypass,
                replica_groups=groups, ins=[a2a_in1.opt()], outs=[a2a_out1.opt()],
            )
            for half, eng in ((0, nc.sync), (1, nc.gpsimd)):
                eng.dma_start(
                    a2a_sb1[:, :, 128 * half:128 * half + 128],
                    a2a_out1[:, :, 128 * half:128 * half + 128]
                    .rearrange("s p m -> p s m"))

            store_engs = (nc.sync, nc.scalar, nc.gpsimd)
            for i8, (qt, nch2) in enumerate(chunks):
                psf = psp.tile([128, CH], F32, tag="ps", bufs=3, name="psfO")
                nsl = slice(CH * nch2, CH * nch2 + CH)
                for i, g in enumerate(odds):
                    op_mm(psf, qt, nsl, g, i == 0, i == NC_CORES - 1)
                osb = wp.tile([128, CH], F32, tag="osb", bufs=2, name="osb")
                nc.vector.tensor_tensor(osb[:], psf[:], partials[:, i8, :], ADD)
                store_engs[i8 % 3].dma_start(out[128 * qt:128 * qt + 128, nsl],
                                             osb[:])

    nc.finalize()
    return nc


def _get_nc():
    if "nc" not in _CACHE:
        _CACHE["nc"] = _build_nc()
    return _CACHE["nc"]


def _shard(inputs):
    import ml_dtypes
    x = np.ascontiguousarray(inputs["x"][0].astype(np.float32))          # [S, D]
    wq, wk, wv = (np.asarray(inputs[k]).astype(np.float32) for k in ("wq", "wk", "wv"))
    wo = np.ascontiguousarray(np.asarray(inputs["wo"]).astype(ml_dtypes.bfloat16))
    cos = np.asarray(inputs["freqs_cos"]).astype(np.float32)   # [S, 32]
    sin = np.asarray(inputs["freqs_sin"]).astype(np.float32)
    # xT layout [128 part, DT, S]: [p, t, s] = x[s, 128 t + p]  (shared)
    xTl = np.ascontiguousarray(
        x.T.reshape(DT, 128, SEQ).transpose(1, 0, 2).astype(ml_dtypes.bfloat16))
    # cosT/sinT [128, S]: row m = freq m%32, replicated x4  (shared)
    cosT = np.ascontiguousarray(
        np.tile(cos.T, (4, 1)).astype(ml_dtypes.bfloat16))
    sinT = np.ascontiguousarray(
        np.tile(sin.T, (4, 1)).astype(ml_dtypes.bfloat16))
    # triangle mask for the diagonal 128x128 block (keep col >= row)
    tri = (np.arange(128)[None, :] >= np.arange(128)[:, None]).astype(np.float32)
    tri2 = np.ascontiguousarray(
        np.broadcast_to(tri[:, None, :], (128, 2, 128)).astype(ml_dtypes.bfloat16))
    # tiled identity [I64 I64]: repk[r, c] = 1 iff c % 64 == r
    repk = np.ascontiguousarray(
        (np.arange(128)[None, :] % 64 == np.arange(64)[:, None])
        .astype(ml_dtypes.bfloat16))

    wq4 = wq.reshape(DIM, 32, 32, 2)       # [d_in, head, freq, r/i]
    wk4 = wk.reshape(DIM, 8, 32, 2)
    wv3 = wv.reshape(DIM, 8, HD)

    def lhsT_tiles(w2d):                   # [2048, 128] -> [128, DT, 128]
        return np.ascontiguousarray(
            w2d.reshape(DT, 128, 128).transpose(1, 0, 2)
            .astype(ml_dtypes.bfloat16))

    in_maps = []
    for c in range(NC_CORES):
        # qR cols m: head 4c + m//32, freq m%32, real part; qI imaginary
        wqR = wq4[:, 4 * c:4 * c + 4, :, 0].reshape(DIM, 128)
        wqI = wq4[:, 4 * c:4 * c + 4, :, 1].reshape(DIM, 128)
        # wkv cols: [v(64) | kr(32) | ki(32)] for kv-head c
        wkvc = np.concatenate([wv3[:, c, :], wk4[:, c, :, 0],
                               wk4[:, c, :, 1]], axis=1)
        in_maps.append({
            "xT": xTl,
            "wqR": lhsT_tiles(wqR),
            "wqI": lhsT_tiles(wqI),
            "wkv": lhsT_tiles(wkvc),
            "wo": wo,
            "cosT": cosT,
            "sinT": sinT,
            "tri2": tri2,
            "repk": repk,
        })
    return in_maps


def kernel(**inputs):
    from concourse.bass_utils import run_bass_kernel_spmd

    nc = _get_nc()
    in_maps = _shard(inputs)
    res = run_bass_kernel_spmd(nc, in_maps, core_ids=list(range(NC_CORES)))
    out = np.concatenate([res.results[c]["out"] for c in range(NC_CORES)], axis=0)
    return out[None].astype(np.float32)


# revision 45
# speedup vs baseline: 1.4039x; 1.0550x over previous
"""Tensor-parallel GQA attention forward for one TRN2 chip (8 NeuronCores).

Strategy (8-way tensor parallel over heads, no pre-attention collectives):
  - each core owns 4 q-heads + 1 kv-head and projects them DIRECTLY in the
    transposed layout from the full x (host supplies xT [128, 16, 2048]
    bf16, streamed in 512-column chunks): out[d, s] = w[:, d]^T @ xT.
    This removes the three pre-attention AllToAlls entirely - the first
    collective in the kernel (attnT redistribution) is not needed until
    attention pair-0 is done, so the multi-rank launch skew that gates the
    first collective costs nothing.
  - q is projected as separate real/imag row-blocks (qR rows = [h0r h1r
    h2r h3r], qI likewise, host-permuted weight columns), so RoPE runs as
    six 128-partition DVE multiplies against host-prepared cosT/sinT
    [freq, seq] tables; k rides the same scheme in a packed [kr ki v]
    projection
  - scores contract r- and i- halves in two accumulating K=32 matmuls per
    head (explicit tile_position row-tiling keeps head pairs concurrent);
    kR/kI are replicated x4 across partition blocks so lhsT/rhs bases line
    up; v is PE-transposed into [s, d] for the PV matmul
  - scores land transposed (S^T[k, q]) in PSUM so exp runs straight out of
    PSUM; softmax denominators come free as ones-columns in the PV matmul;
    causal masking = skipping k-tiles above the diagonal, a column
    trapezoid on the diagonal band, and a 128-wide triangle multiply
  - attention pair-0 interleaves with the tail of the projections and
    pair-1 interleaves with the even half of the output projection, so the
    PE stays dense (HAM stays at 2.4 GHz) through the ACT-paced softmax
  - an AllToAll flips head-sharded attnT to sequence-sharded; remaining
    even-half groups fill the final collective's window, then the odd half
    runs and the halves are summed
  - compute dtype bf16 (fp32 PSUM accumulation), output fp32
"""

import numpy as np

NC_CORES = 8
SEQ = 2048
DIM = 2048
HD = 64            # head dim
SC = SEQ // NC_CORES   # 256: sequence rows per core (output shard)
CH = 512           # q-chunk width for attention / projection s-chunk
NCH = SEQ // CH    # 4
KT = SEQ // 128    # 16 k-tiles
DT = DIM // 128    # 16 d-tiles

_CACHE = {}


def _build_nc():
    import concourse.bass as bass
    import concourse.mybir as mybir
    import concourse.tile as tile
    from concourse import bacc
    from concourse.masks import make_identity

    BF = mybir.dt.bfloat16
    F32 = mybir.dt.float32
    MUL = mybir.AluOpType.mult
    ADD = mybir.AluOpType.add
    SUB = mybir.AluOpType.subtract

    nc = bacc.Bacc("TRN2", target_bir_lowering=False, debug=False,
                   num_devices=NC_CORES)

    # ---- external I/O (per-core shards) ----
    xT = nc.dram_tensor("xT", [128, DT, SEQ], BF, kind="ExternalInput")
    wqR = nc.dram_tensor("wqR", [128, DT, 128], BF, kind="ExternalInput")
    wqI = nc.dram_tensor("wqI", [128, DT, 128], BF, kind="ExternalInput")
    wkv = nc.dram_tensor("wkv", [128, DT, 128], BF, kind="ExternalInput")
    wo = nc.dram_tensor("wo", [DIM, DIM], BF, kind="ExternalInput")
    cosT_in = nc.dram_tensor("cosT", [128, SEQ], BF, kind="ExternalInput")
    sinT_in = nc.dram_tensor("sinT", [128, SEQ], BF, kind="ExternalInput")
    repk_in = nc.dram_tensor("repk", [64, 128], BF, kind="ExternalInput")
    tri2_in = nc.dram_tensor("tri2", [128, 2, 128], BF, kind="ExternalInput")
    out = nc.dram_tensor("out", [SC, DIM], F32, kind="ExternalOutput")

    groups = [list(range(NC_CORES))]

    with tile.TileContext(nc) as tc:
        # DRAM bounce buffers for the output-side collectives
        a2a_in0, _ = tc.tile([NC_CORES, 128, SC], BF,
                             space=bass.MemorySpace.DRAM, name="a2a_in0")
        a2a_out0, _ = tc.tile([NC_CORES, 128, SC], BF,
                              space=bass.MemorySpace.DRAM,
                              addr_space="Shared", name="a2a_out0")
        a2a_in1, _ = tc.tile([NC_CORES, 128, SC], BF,
                             space=bass.MemorySpace.DRAM, name="a2a_in1")
        a2a_out1, _ = tc.tile([NC_CORES, 128, SC], BF,
                              space=bass.MemorySpace.DRAM,
                              addr_space="Shared", name="a2a_out1")

        with tc.tile_pool(name="persist", bufs=1) as pp, \
             tc.tile_pool(name="work", bufs=2) as wp, \
             tc.tile_pool(name="psum", bufs=2, space="PSUM") as psp:

            # weights (small: 1.5 MB total) and tables load first
            wq_sb = pp.tile([128, 2, DT, 128], BF, name="wq_sb")
            nc.sync.dma_start(wq_sb[:, 0, :, :], wqR[:])
            nc.sync.dma_start(wq_sb[:, 1, :, :], wqI[:])
            wkv_sb = pp.tile([128, DT, 128], BF, name="wkv_sb")
            nc.sync.dma_start(wkv_sb[:], wkv[:])
            cosT = pp.tile([128, SEQ], BF, name="cosT")
            sinT = pp.tile([128, SEQ], BF, name="sinT")
            nc.sync.dma_start(cosT[:], cosT_in[:])
            nc.sync.dma_start(sinT[:], sinT_in[:])
            tri2 = pp.tile([128, 2, 128], BF, name="tri2")
            nc.sync.dma_start(tri2[:], tri2_in[:])
            repk_sb = pp.tile([64, 128], BF, name="repk_sb")
            nc.sync.dma_start(repk_sb[:], repk_in[:])
            ident = pp.tile([128, 128], BF, name="ident")
            make_identity(nc, ident[:])

            # roped q per pair: rows = [h0r(32) h0i(32) h1r(32) h1i(32)]
            qP = [pp.tile([128, SEQ], BF, name=f"qP{p}") for p in range(2)]
            # roped k, [kr ki] duplicated: rows = [kr ki kr ki]
            kT2 = pp.tile([128, SEQ], BF, name="kT2")
            v_sb = pp.tile([128, KT, 2 * HD], BF, name="v_sb")
            nc.gpsimd.memset(v_sb[:, :, HD:2 * HD], 1.0)

            # ---------------- direct projections, one 512-seq chunk at a time
            # Each chunk is built as a list of small steps (~4 matmuls each)
            # so it can drip into the attention loop as PE filler.
            def chunk_steps(sc):
                ssl = slice(CH * sc, CH * sc + CH)
                state = {}
                steps = []

                def load():
                    xc = wp.tile([128, DT, CH], BF, tag="xring", bufs=2,
                                 name="xc")
                    e0 = nc.scalar if sc % 2 == 0 else nc.gpsimd
                    e1 = nc.gpsimd if sc % 2 == 0 else nc.scalar
                    e0.dma_start(xc[:, 0:4, :], xT[:, 0:4, ssl])
                    e1.dma_start(xc[:, 4:8, :], xT[:, 4:8, ssl])
                    e0.dma_start(xc[:, 8:12, :], xT[:, 8:12, ssl])
                    e1.dma_start(xc[:, 12:16, :], xT[:, 12:16, ssl])
                    state["xc"] = xc
                steps.append(load)

                # packed [v(64) | kr(32) | ki(32)] projection
                def kv_mms(i0):
                    def f():
                        if i0 == 0:
                            state["pkv"] = psp.tile([128, CH], F32, tag="ps",
                                                    bufs=3, name="pkv")
                        for dt in range(i0, i0 + 4):
                            nc.tensor.matmul(
                                state["pkv"][:], wkv_sb[:, dt, :],
                                state["xc"][:, dt, :],
                                start=(dt == 0), stop=(dt == DT - 1))
                    return f
                for i0 in range(0, DT, 4):
                    steps.append(kv_mms(i0))

                def kv_post():
                    kvraw = wp.tile([128, CH], BF, tag="kvraw", bufs=2,
                                    name="kvraw")
                    nc.scalar.copy(kvraw[:], state["pkv"][:])
                    state["kvraw"] = kvraw
                    # k RoPE on rows 64:128 (cosT's freq pattern repeats
                    # every 32 rows - read the rows matching each input's
                    # partitions; DVE TT needs equal input base partitions)
                    kro = wp.tile([64, CH], BF, tag="kro", bufs=2,
                                  name="kro")
                    kr, ki = kvraw[64:96, :], kvraw[96:128, :]
                    ta = wp.tile([32, 2, CH], BF, tag="kta", bufs=2, name="kta")
                    nc.vector.tensor_tensor(ta[:, 0, :], kr,
                                            cosT[64:96, ssl], MUL)
                    nc.vector.tensor_tensor(ta[:, 1, :], ki,
                                            sinT[96:128, ssl], MUL)
                    nc.vector.tensor_tensor(kro[0:32, :], ta[:, 0, :],
                                            ta[:, 1, :], SUB)
                    nc.vector.tensor_tensor(ta[:, 0, :], kr,
                                            sinT[64:96, ssl], MUL)
                    nc.vector.tensor_tensor(ta[:, 1, :], ki,
                                            cosT[96:128, ssl], MUL)
                    nc.vector.tensor_tensor(kro[32:64, :], ta[:, 0, :],
                                            ta[:, 1, :], ADD)
                    state["kro"] = kro
                steps.append(kv_post)

                # duplicate roped [kr ki] into rows [kr ki kr ki] via a
                # tiled-identity matmul (lane-correct partition broadcast)
                def k_rep():
                    prep = psp.tile([128, CH], F32, tag="ps", bufs=3,
                                    name="prep")
                    nc.tensor.matmul(prep[:], repk_sb[:], state["kro"][:],
                                     start=True, stop=True)
                    nc.vector.tensor_copy(kT2[:, ssl], prep[:])
                steps.append(k_rep)

                # v: PE-transpose [128, 128] blocks; keep the v rows
                def v_tr(i):
                    def f():
                        kt = 4 * sc + i
                        tv = psp.tile([128, 128], BF, tag="ps", bufs=3,
                                      name="tv")
                        nc.tensor.transpose(tv[:],
                                            state["kvraw"][:, 128 * i:128 * i + 128],
                                            ident[:])
                        nc.vector.tensor_copy(v_sb[:, kt, 0:HD],
                                              tv[0:128, 0:64])
                    return f
                for i in range(4):
                    steps.append(v_tr(i))

                # q r/i projections
                def q_mms(ri, i0):
                    def f():
                        if i0 == 0:
                            state[f"pq{ri}"] = psp.tile([128, CH], F32,
                                                        tag="ps", bufs=3,
                                                        name=f"pq{ri}")
                        for dt in range(i0, i0 + 4):
                            nc.tensor.matmul(
                                state[f"pq{ri}"][:], wq_sb[:, ri, dt, :],
                                state["xc"][:, dt, :],
                                start=(dt == 0), stop=(dt == DT - 1))
                    return f

                def q_post(ri):
                    def f():
                        qraw = wp.tile([128, CH], BF, tag=f"qraw{ri}", bufs=2,
                                       name=f"qraw{ri}")
                        nc.scalar.copy(qraw[:], state[f"pq{ri}"][:])
                        state[f"qraw{ri}"] = qraw
                    return f
                for ri in range(2):
                    for i0 in range(0, DT, 4):
                        steps.append(q_mms(ri, i0))
                    steps.append(q_post(ri))

                def q_rope():
                    # products on all 128 rows at once, then per-head 32-row
                    # writes into the pair tiles' [hr | hi] row layout
                    ct4, st4 = cosT[:, ssl], sinT[:, ssl]
                    qr, qi = state["qraw0"][:], state["qraw1"][:]
                    tq = wp.tile([128, 2, CH], BF, tag="qta", bufs=2, name="tq")
                    nc.vector.tensor_tensor(tq[:, 0, :], qr, ct4, MUL)
                    nc.vector.tensor_tensor(tq[:, 1, :], qi, st4, MUL)
                    for g in range(4):
                        rs = slice(32 * g, 32 * g + 32)
                        dst = qP[g // 2][64 * (g % 2):64 * (g % 2) + 32, ssl]
                        nc.vector.tensor_tensor(dst, tq[rs, 0, :],
                                                tq[rs, 1, :], SUB)
                    nc.vector.tensor_tensor(tq[:, 0, :], qr, st4, MUL)
                    nc.vector.tensor_tensor(tq[:, 1, :], qi, ct4, MUL)
                    for g in range(4):
                        rs = slice(32 * g, 32 * g + 32)
                        dst = qP[g // 2][64 * (g % 2) + 32:64 * (g % 2) + 64,
                                         ssl]
                        nc.vector.tensor_tensor(dst, tq[rs, 0, :],
                                                tq[rs, 1, :], ADD)
                steps.append(q_rope)
                return steps

            drip = {"steps": []}

            def drip_run(n):
                for _ in range(n):
                    if not drip["steps"]:
                        return
                    drip["steps"].pop(0)()

            def drip_flush():
                drip_run(len(drip["steps"]) + 1)

            # ---------------- attention ----------------
            attnT = pp.tile([128, 2, SEQ], BF, name="attnT")

            def attention(pair, j, interleave=None):
                nkt = 4 * j + 4
                nfull = nkt - 4
                pso0 = psp.tile([2 * HD, CH], F32, tag="ps", bufs=3, name="pso0")
                pso1 = psp.tile([2 * HD, CH], F32, tag="ps", bufs=3, name="pso1")
                qsl = slice(CH * j, CH * j + CH)
                qPt = qP[pair]

                def pv(ep_h0, ep_h1, kt, c0):
                    nc.tensor.matmul(pso0[:, c0:CH], v_sb[:, kt, :], ep_h0,
                                     start=(kt == 0), stop=(kt == nkt - 1))
                    nc.tensor.matmul(pso1[:, c0:CH], v_sb[:, kt, :], ep_h1,
                                     start=(kt == 0), stop=(kt == nkt - 1))

                # software pipeline: PV of k-tile i runs one iteration behind
                # its exp, with the drip filler issued in between on the PE
                prev = None
                for kt in range(nkt):
                    t = kt - nfull            # >= 0 on the diagonal band
                    c0 = 128 * t if t >= 0 else 0
                    ks = slice(128 * kt, 128 * kt + 128)
                    qs = slice(CH * j + c0, CH * j + CH)
                    sp = psp.tile([128, 2, CH], F32, tag="sp2", bufs=2,
                                  name="sp")
                    for h in range(2):
                        nc.tensor.matmul(sp[:, h, c0:CH],
                                         kT2[64 * h:64 * h + 64, ks],
                                         qPt[64 * h:64 * h + 64, qs],
                                         start=True, stop=True)
                    ep = wp.tile([128, 2, CH], BF, tag="exps", bufs=4,
                                 name="ep")
                    nc.scalar.activation(ep[:, :, c0:CH], sp[:, :, c0:CH],
                                         mybir.ActivationFunctionType.Exp,
                                         scale=0.125)
                    if t >= 0:
                        nc.vector.tensor_tensor(ep[:, :, c0:c0 + 128],
                                                ep[:, :, c0:c0 + 128],
                                                tri2[:], MUL)
                    if interleave is not None:
                        interleave(j, kt)
                    if prev is not None:
                        pv(*prev)
                    prev = (ep[:, 0, c0:CH], ep[:, 1, c0:CH], kt, c0)
                pv(*prev)

                for h, pso in ((0, pso0), (1, pso1)):
                    bc = wp.tile([64, CH], F32, tag="bcast", bufs=2, name="bc")
                    nc.vector.tensor_copy(bc[:], pso[HD:2 * HD, :])
                    rc = wp.tile([64, CH], F32, tag="rcp", bufs=2, name="rc")
                    nc.vector.reciprocal_approx_fast(out=rc[:], in_=bc[:])
                    nc.vector.tensor_tensor(
                        attnT[64 * h:64 * h + 64, pair, qsl],
                        pso[0:HD, :], rc[:], MUL)

            # ---------------- output projection helpers ----------------
            woA = pp.tile([128, DT // 2, DIM], BF, name="woA")
            woB = pp.tile([128, DT // 2, DIM], BF, name="woB")
            a2a_sb0 = pp.tile([128, NC_CORES, SC], BF, name="a2a_sb0")
            a2a_sb1 = pp.tile([128, NC_CORES, SC], BF, name="a2a_sb1")
            partials = pp.tile([128, 2 * NCH, CH], BF, tag="proj",
                               name="partials")
            evens = [2 * src for src in range(NC_CORES)]
            odds = [2 * src + 1 for src in range(NC_CORES)]
            chunks = [(qt, nch) for qt in range(2) for nch in range(NCH)]

            def op_mm(psf, qt, nsl, g, start, stop):
                w_ap = (woA[:, g, nsl] if g < DT // 2
                        else woB[:, g - DT // 2, nsl])
                a_ap = (a2a_sb0[:, g // 2, 128 * qt:128 * qt + 128] if g % 2 == 0
                        else a2a_sb1[:, g // 2, 128 * qt:128 * qt + 128])
                nc.tensor.matmul(psf[:], a_ap, w_ap, start=start, stop=stop)

            ev_state = {"psf": None, "n": 0}

            def even_steps(nsteps):
                for _ in range(nsteps):
                    n = ev_state["n"]
                    if n >= 64:
                        return
                    i8, i = divmod(n, NC_CORES)
                    qt, nch2 = chunks[i8]
                    if i == 0:
                        # sp2 slots are idle once attention is done - two
                        # buffers let consecutive even groups pipeline past
                        # the partials copy
                        ev_state["psf"] = psp.tile([128, CH], F32, tag="sp2",
                                                   bufs=2, name="psfE")
                    nsl = slice(CH * nch2, CH * nch2 + CH)
                    op_mm(ev_state["psf"], qt, nsl, evens[i],
                          i == 0, i == NC_CORES - 1)
                    if i == NC_CORES - 1:
                        nc.vector.tensor_copy(partials[:, i8, :],
                                              ev_state["psf"][:])
                    ev_state["n"] = n + 1

            # ---------------- pair-0 attention, proj-interleaved ----------
            # proj chunk c+1 drips into attention chunk c as PE filler and
            # is flushed before the attention chunk that first needs it
            for f in chunk_steps(0):
                f()
            drip_rate = (5, 3, 2, 0)

            def inter0(jj, kt):
                drip_run(drip_rate[jj])

            for j in range(NCH):
                if j < NCH - 1:
                    drip["steps"] = chunk_steps(j + 1)
                attention(0, j, interleave=inter0)
                drip_flush()
                nc.scalar.dma_start(
                    a2a_in0[2 * j:2 * j + 2, :, :]
                    .rearrange("d p m -> p d m"),
                    attnT[:, 0, CH * j:CH * j + CH]
                    .rearrange("p (d m) -> p d m", m=SC))
                # anchored wo prefetch (the scheduler hoists dep-free DMAs)
                nc.vector.tensor_copy(woA[0:1, 2 * j, 0:1],
                                      attnT[0:1, 0, CH * j:CH * j + 1])
                nc.sync.dma_start(
                    woA[:, 2 * j:2 * j + 2, :],
                    wo[256 * j:256 * j + 256, :].rearrange("(t p) n -> p t n",
                                                           p=128))
                if j >= 2:   # woB too: needed by the interleaved even groups
                    jb = j - 2
                    nc.vector.tensor_copy(woB[0:1, 4 * jb, 0:1],
                                          attnT[0:1, 0, CH * j:CH * j + 1])
                    nc.gpsimd.dma_start(
                        woB[:, 4 * jb:4 * jb + 4, :],
                        wo[1024 + 512 * jb:1024 + 512 * jb + 512, :]
                        .rearrange("(t p) n -> p t n", p=128))
            nc.gpsimd.collective_compute(
                "AllToAll", mybir.AluOpType.bypass,
                replica_groups=groups, ins=[a2a_in0.opt()], outs=[a2a_out0.opt()],
            )
            for half in range(2):
                nc.sync.dma_start(
                    a2a_sb0[:, :, 128 * half:128 * half + 128],
                    a2a_out0[:, :, 128 * half:128 * half + 128]
                    .rearrange("s p m -> p s m"))

            # ---------------- pair-1 attention (pure) ----------------
            # Chunk order 1,0,2,3: chunk 0 is all-diagonal (tri-mult-gated on
            # the DVE) and would stall right behind pair-0's normalize chain.
            # No interleave: pair-1 finishes (and the final A2A triggers) as
            # early as possible; the even outproj then fills the A2A window.
            p1_order = (1, 0, 2, 3)
            for j in p1_order:
                attention(1, j)
                nc.scalar.dma_start(
                    a2a_in1[2 * j:2 * j + 2, :, :]
                    .rearrange("d p m -> p d m"),
                    attnT[:, 1, CH * j:CH * j + CH]
                    .rearrange("p (d m) -> p d m", m=SC))

            # ---------------- final A2A + remaining outproj ----------------
            # evens are emitted BEFORE the collective call so tile's block
            # ordering doesn't gate them behind it - they only depend on
            # a2a_sb0, so they execute inside the collective's skew + wire
            # window on the PE
            even_steps(64)
            nc.gpsimd.collective_compute(
                "AllToAll", mybir.AluOpType.bypass,
                replica_groups=groups, ins=[a2a_in1.opt()], outs=[a2a_out1.opt()],
            )
            for half, eng in ((0, nc.sync), (1, nc.gpsimd)):
                eng.dma_start(
                    a2a_sb1[:, :, 128 * half:128 * half + 128],
                    a2a_out1[:, :, 128 * half:128 * half + 128]
                    .rearrange("s p m -> p s m"))

            store_engs = (nc.sync, nc.scalar, nc.gpsimd)
            for i8, (qt, nch2) in enumerate(chunks):
                psf = psp.tile([128, CH], F32, tag="ps", bufs=3, name="psfO")
                nsl = slice(CH * nch2, CH * nch2 + CH)
                for i, g in enumerate(odds):
                    op_mm(psf, qt, nsl, g, i == 0, i == NC_CORES - 1)
                osb = wp.tile([128, CH], F32, tag="osb", bufs=2, name="osb")
                nc.vector.tensor_tensor(osb[:], psf[:], partials[:, i8, :], ADD)
                store_engs[i8 % 3].dma_start(out[128 * qt:128 * qt + 128, nsl],
                                             osb[:])

    nc.finalize()
    return nc


def _get_nc():
    if "nc" not in _CACHE:
        _CACHE["nc"] = _build_nc()
    return _CACHE["nc"]


def _shard(inputs):
    import ml_dtypes
    x = np.ascontiguousarray(inputs["x"][0].astype(np.float32))          # [S, D]
    wq, wk, wv = (np.asarray(inputs[k]).astype(np.float32) for k in ("wq", "wk", "wv"))
    wo = np.ascontiguousarray(np.asarray(inputs["wo"]).astype(ml_dtypes.bfloat16))
    cos = np.asarray(inputs["freqs_cos"]).astype(np.float32)   # [S, 32]
    sin = np.asarray(inputs["freqs_sin"]).astype(np.float32)
    # xT layout [128 part, DT, S]: [p, t, s] = x[s, 128 t + p]  (shared)
    xTl = np.ascontiguousarray(
        x.T.reshape(DT, 128, SEQ).transpose(1, 0, 2).astype(ml_dtypes.bfloat16))
    # cosT/sinT [128, S]: row m = freq m%32, replicated x4  (shared)
    cosT = np.ascontiguousarray(
        np.tile(cos.T, (4, 1)).astype(ml_dtypes.bfloat16))
    sinT = np.ascontiguousarray(
        np.tile(sin.T, (4, 1)).astype(ml_dtypes.bfloat16))
    # triangle mask for the diagonal 128x128 block (keep col >= row)
    tri = (np.arange(128)[None, :] >= np.arange(128)[:, None]).astype(np.float32)
    tri2 = np.ascontiguousarray(
        np.broadcast_to(tri[:, None, :], (128, 2, 128)).astype(ml_dtypes.bfloat16))
    # tiled identity [I64 I64]: repk[r, c] = 1 iff c % 64 == r
    repk = np.ascontiguousarray(
        (np.arange(128)[None, :] % 64 == np.arange(64)[:, None])
        .astype(ml_dtypes.bfloat16))

    wq4 = wq.reshape(DIM, 32, 32, 2)       # [d_in, head, freq, r/i]
    wk4 = wk.reshape(DIM, 8, 32, 2)
    wv3 = wv.reshape(DIM, 8, HD)

    def lhsT_tiles(w2d):                   # [2048, 128] -> [128, DT, 128]
        return np.ascontiguousarray(
            w2d.reshape(DT, 128, 128).transpose(1, 0, 2)
            .astype(ml_dtypes.bfloat16))

    in_maps = []
    for c in range(NC_CORES):
        # qR cols m: head 4c + m//32, freq m%32, real part; qI imaginary
        wqR = wq4[:, 4 * c:4 * c + 4, :, 0].reshape(DIM, 128)
        wqI = wq4[:, 4 * c:4 * c + 4, :, 1].reshape(DIM, 128)
        # wkv cols: [v(64) | kr(32) | ki(32)] for kv-head c
        wkvc = np.concatenate([wv3[:, c, :], wk4[:, c, :, 0],
                               wk4[:, c, :, 1]], axis=1)
        in_maps.append({
            "xT": xTl,
            "wqR": lhsT_tiles(wqR),
            "wqI": lhsT_tiles(wqI),
            "wkv": lhsT_tiles(wkvc),
            "wo": wo,
            "cosT": cosT,
            "sinT": sinT,
            "tri2": tri2,
            "repk": repk,
        })
    return in_maps


def kernel(**inputs):
    from concourse.bass_utils import run_bass_kernel_spmd

    nc = _get_nc()
    in_maps = _shard(inputs)
    res = run_bass_kernel_spmd(nc, in_maps, core_ids=list(range(NC_CORES)))
    out = np.concatenate([res.results[c]["out"] for c in range(NC_CORES)], axis=0)
    return out[None].astype(np.float32)
